# revision 1
# baseline (speedup 1.0000x reference)
"""Two-layer GCN (GCNConv x2, PyG-style symmetric normalization) on 8 trn2
NeuronCores.

Strategy (vertex-cut graph parallelism):
  - Nodes are sharded into 8 contiguous buckets (padded to a multiple of
    128).  Core c owns bucket c: it computes the dense transform for its
    rows and aggregates all edges whose *destination* falls in its bucket.
  - Normalization is factored:  out[d] = b + dis[d]*(sum_{e:col=d} g[row_e]
    + g[d]),  g[n] = dis[n]*(x@W)[n],  dis = 1/sqrt(deg).  This removes all
    per-edge multiplies: aggregation is a pure indicator matmul.
  - Per core the bf16 "g" shard is exchanged via 4 piece-wise AllGathers so
    gathers can start before the full table has arrived.
  - Edge aggregation: edges sorted by destination block; per 128-edge tile
    a one-hot mask (edges x 128 dst slots) is built on the vector engine by
    comparing an iota row against the tile's dst values; the TensorEngine
    accumulates mask.T @ gathered_rows into the block's PSUM accumulator.
    Source rows are fetched with the SWDGE dma_gather instruction (int16
    indices, which is why the table is split into 4 chunks).
  - Bias is pre-seeded into PSUM as a K=1 outer product sqrt(deg)[d]*b[ch]
    so the final per-block eviction is a single fused  (psum*dis, relu)
    tensor_scalar op.  Self-loops are one identity matmul per block.

Host-side work is limited to index plumbing: bucketing/sorting edges,
building gather-index/mask-value planes, degree counts (a byproduct of the
destination bucketing) and data layout (transpose/pad).  All floating-point
math (1/sqrt, matmuls, scaling, bias, relu) runs on device.
"""

import math
import os

import numpy as np

DBG_NO_COLL = os.environ.get("GCN_NO_COLL", "0") == "1"
DBG_NO_GATHER = os.environ.get("GCN_NO_GATHER", "0") == "1"

CFG_FULL = dict(N=100000, E=1600000, CIN=128, CHID=128, COUT=64)

NCORES = 8
PIECES = 4  # table pieces / AllGather splits
SUPER = 8  # dst blocks per gather batch group


def _derive(cfg):
    n = cfg["N"]
    bucket = n // NCORES
    assert bucket * NCORES == n
    blocks = math.ceil(bucket / 128)
    blocks = math.ceil(blocks / PIECES) * PIECES
    shard = blocks * 128
    qrows = shard // PIECES  # rows per piece per core
    chunk = qrows * NCORES  # rows of one assembled table piece
    assert chunk <= 32600, chunk  # int16 gather index limit
    supers = [SUPER] * (blocks // SUPER)
    if blocks % SUPER:
        supers.append(blocks % SUPER)
    return dict(bucket=bucket, blocks=blocks, shard=shard, qrows=qrows,
                chunk=chunk, supers=supers)


def _preprocess(edge_index, cfg):
    """Bucket & sort edges, build per-core gather/mask planes."""
    d = _derive(cfg)
    bucket, blocks, qrows = d["bucket"], d["blocks"], d["qrows"]
    row = edge_index[0].astype(np.int64)
    col = edge_index[1].astype(np.int64)

    c_dst = col // bucket
    d_l = col - c_dst * bucket
    blk = d_l // 128
    rel = (d_l % 128).astype(np.float32)
    c_src = row // bucket
    r_l = row - c_src * bucket
    q = r_l // qrows
    ric = (c_src * qrows + r_l % qrows).astype(np.int64)  # row in chunk q

    nbq = blocks * PIECES
    key_bq = blk * PIECES + q
    counts = np.zeros((NCORES, nbq), np.int64)
    for c in range(NCORES):
        m = c_dst == c
        counts[c] = np.bincount(key_bq[m], minlength=nbq)
    tiles_bq = np.ceil(counts.max(axis=0) / 128).astype(np.int64)  # [nbq]

    # tile schedule in program order: (super, piece, block in super, tile)
    order_bq = []
    supers = d["supers"]
    b0 = 0
    batches = []  # tiles per (super, piece) gather batch
    for g in supers:
        for qq in range(PIECES):
            nt = 0
            for b in range(b0, b0 + g):
                order_bq.append((b, qq))
                nt += int(tiles_bq[b * PIECES + qq])
            batches.append(nt)
        b0 += g
    tot_tiles = int(tiles_bq.sum())
    assert sum(batches) == tot_tiles and tot_tiles > 0

    off_bq = np.zeros(nbq, np.int64)
    acc = 0
    for (b, qq) in order_bq:
        off_bq[b * PIECES + qq] = acc
        acc += int(tiles_bq[b * PIECES + qq])

    per_core = []
    for c in range(NCORES):
        m = c_dst == c
        okey = (blk[m] * PIECES + q[m]).astype(np.int64)
        sort = np.argsort(okey, kind="stable")
        okey_s = okey[sort]
        e_rel = rel[m][sort]
        e_ric = ric[m][sort]
        slot_base = off_bq[okey_s] * 128
        grp_start = np.searchsorted(okey_s, okey_s)
        within = np.arange(okey_s.size) - grp_start
        slots = slot_base + within
        idx_flat = np.zeros(tot_tiles * 128, np.int16)
        rel_flat = np.full(tot_tiles * 128, -1.0, np.float32)
        idx_flat[slots] = e_ric.astype(np.int16)
        rel_flat[slots] = e_rel
        idx16 = idx_flat.reshape(tot_tiles * 8, 16).T  # [16, tiles*8]
        idx_plane = np.tile(idx16, (8, 1)).copy()
        rel_plane = np.ascontiguousarray(
            rel_flat.reshape(tot_tiles, 128).T)  # [128, tot_tiles]
        per_core.append(dict(idx_plane=idx_plane, rel_plane=rel_plane))

    meta = dict(d=d, tiles_bq=tiles_bq, batches=batches, tot_tiles=tot_tiles,
                supers=supers)
    return meta, per_core


def _host_inputs(x, edge_index, W1, b1, W2, b2, cfg):
    d = _derive(cfg)
    bucket, blocks, shard = d["bucket"], d["blocks"], d["shard"]
    n, cin = cfg["N"], cfg["CIN"]
    chid, cout = cfg["CHID"], cfg["COUT"]
    meta, per_core = _preprocess(edge_index, cfg)

    col = edge_index[1].astype(np.int64)
    deg = (np.bincount(col, minlength=n) + 1).astype(np.float32)

    w1 = np.ascontiguousarray(np.asarray(W1, np.float32))
    w2p = np.zeros((chid, 128), np.float32)
    w2p[:, :cout] = np.asarray(W2, np.float32)
    b1d = np.zeros((4, 512), np.float32)
    b2d = np.zeros((4, 512), np.float32)
    for k in range(4):
        b1d[k, k * 128:k * 128 + chid] = np.asarray(b1, np.float32)
        b2d[k, k * 128:k * 128 + cout] = np.asarray(b2, np.float32)
    iota = np.ascontiguousarray(
        np.broadcast_to(np.arange(128, dtype=np.float32)[None, :],
                        (128, 128)))
    eye = np.eye(128, dtype=np.float32)

    in_maps = []
    for c in range(NCORES):
        xs = np.zeros((shard, cin), np.float32)
        xs[:bucket] = x[c * bucket:(c + 1) * bucket]
        x_ct = np.ascontiguousarray(xs.T)  # [cin, shard]
        degs = np.ones(shard, np.float32)
        degs[:bucket] = deg[c * bucket:(c + 1) * bucket]
        deg_pm = np.ascontiguousarray(degs.reshape(blocks, 128).T)
        # [4, nbanks*128]: [k, g*128+p] = deg[(4g+k)*128+p]
        deg_b4 = np.ascontiguousarray(
            degs.reshape(blocks // 4, 4, 128).transpose(1, 0, 2)
            .reshape(4, -1))
        in_maps.append({
            "x_ct": x_ct, "deg_pm": deg_pm, "deg_b4": deg_b4,
            "idx_plane": per_core[c]["idx_plane"],
            "rel_plane": per_core[c]["rel_plane"],
            "w1": w1, "w2p": w2p, "b1d": b1d, "b2d": b2d,
            "iota": iota, "eye": eye,
        })
    return meta, in_maps


def _build_program(cfg, meta):
    import concourse.bacc as bacc
    import concourse.mybir as mybir
    from concourse import tile

    d = meta["d"]
    blocks, shard, qrows, chunk = (d["blocks"], d["shard"], d["qrows"],
                                   d["chunk"])
    supers = meta["supers"]
    tiles_bq = meta["tiles_bq"]
    tot_tiles = meta["tot_tiles"]
    batches = meta["batches"]
    cin, chid, cout = cfg["CIN"], cfg["CHID"], cfg["COUT"]
    bpp = blocks // PIECES  # blocks per piece

    bf16 = mybir.dt.bfloat16
    f32 = mybir.dt.float32
    i16 = mybir.dt.int16
    mult = mybir.AluOpType.mult
    amax = mybir.AluOpType.max
    iseq = mybir.AluOpType.is_equal

    nc = bacc.Bacc("TRN2", target_bir_lowering=False, debug=False,
                   num_devices=NCORES)

    x_ct = nc.dram_tensor("x_ct", [cin, shard], f32, kind="ExternalInput")
    deg_pm_t = nc.dram_tensor("deg_pm", [128, blocks], f32,
                              kind="ExternalInput")
    deg_b4_t = nc.dram_tensor("deg_b4", [4, (blocks // 4) * 128], f32,
                              kind="ExternalInput")
    idxp_t = nc.dram_tensor("idx_plane", [128, tot_tiles * 8], i16,
                            kind="ExternalInput")
    relp_t = nc.dram_tensor("rel_plane", [128, tot_tiles], f32,
                            kind="ExternalInput")
    w1_t = nc.dram_tensor("w1", [cin, chid], f32, kind="ExternalInput")
    w2p_t = nc.dram_tensor("w2p", [chid, 128], f32, kind="ExternalInput")
    b1d_t = nc.dram_tensor("b1d", [4, 512], f32, kind="ExternalInput")
    b2d_t = nc.dram_tensor("b2d", [4, 512], f32, kind="ExternalInput")
    iota_t = nc.dram_tensor("iota", [128, 128], f32, kind="ExternalInput")
    eye_t = nc.dram_tensor("eye", [128, 128], f32, kind="ExternalInput")
    out_t = nc.dram_tensor("out", [shard, cout], f32, kind="ExternalOutput")

    with tile.TileContext(nc) as tc:
        with (
            tc.tile_pool(name="dram", bufs=1, space="DRAM") as dram,
            tc.tile_pool(name="const", bufs=1) as cp,
            tc.tile_pool(name="shards", bufs=1) as shp,
            tc.tile_pool(name="stage", bufs=2) as stp,
            tc.tile_pool(name="masks", bufs=8) as mp,
            tc.tile_pool(name="work", bufs=4) as wp,
            tc.tile_pool(name="outp", bufs=2) as op_,
            tc.tile_pool(name="pbig", bufs=4, space="PSUM") as pbig,
            tc.tile_pool(name="pph1", bufs=2, space="PSUM") as pph1,
            tc.tile_pool(name="pptr", bufs=1, space="PSUM") as pptr,
            tc.tile_pool(name="ppg", bufs=1, space="PSUM") as ppg,
        ):
            # ---- DRAM scratch ----
            bounce1 = [dram.tile([qrows, chid], bf16, name=f"bo1_{j}",
                                 tag=f"bo1_{j}") for j in range(PIECES)]
            bounce2 = [dram.tile([qrows, 128], bf16, name=f"bo2_{j}",
                                 tag=f"bo2_{j}") for j in range(PIECES)]
            tab1 = [dram.tile([chunk, chid], bf16, name=f"t1_{j}",
                              tag=f"t1_{j}") for j in range(PIECES)]
            tab2 = [dram.tile([chunk, 128], bf16, name=f"t2_{j}",
                              tag=f"t2_{j}") for j in range(PIECES)]

            # ---- constants ----
            iota_sb = cp.tile([128, 128], bf16)
            nc.gpsimd.dma_start(iota_sb[:], iota_t[:])  # cast f32->bf16
            eye_sb = cp.tile([128, 128], bf16)
            nc.gpsimd.dma_start(eye_sb[:], eye_t[:])
            w1_sb = cp.tile([cin, chid], bf16)
            nc.gpsimd.dma_start(w1_sb[:], w1_t[:])
            w2_sb = cp.tile([chid, 128], bf16)
            nc.gpsimd.dma_start(w2_sb[:], w2p_t[:])
            b1_sb = cp.tile([4, 512], f32)
            nc.sync.dma_start(b1_sb[:], b1d_t[:])
            b2_sb = cp.tile([4, 512], f32)
            nc.sync.dma_start(b2_sb[:], b2d_t[:])
            idxp_sb = cp.tile([128, tot_tiles * 8], i16)
            nc.sync.dma_start(idxp_sb[:], idxp_t[:])
            relp_sb = cp.tile([128, tot_tiles], f32)
            nc.sync.dma_start(relp_sb[:], relp_t[:])

            deg_pm = cp.tile([128, blocks], f32)
            nc.sync.dma_start(deg_pm[:], deg_pm_t[:])
            deg_b4 = cp.tile([4, (blocks // 4) * 128], f32)
            nc.sync.dma_start(deg_b4[:], deg_b4_t[:])
            invd_pm = cp.tile([128, blocks], f32)
            nc.scalar.sqrt(invd_pm[:], deg_pm[:])
            dis_pm = cp.tile([128, blocks], f32)
            nc.vector.reciprocal(dis_pm[:], invd_pm[:])
            invd_b4 = cp.tile([4, (blocks // 4) * 128], f32)
            nc.scalar.sqrt(invd_b4[:], deg_b4[:])

            x_sb = shp.tile([cin, shard], bf16)
            nc.gpsimd.dma_start(x_sb[:], x_ct[:])  # cast f32->bf16
            g1s = shp.tile([128, blocks * chid], bf16)
            g2s = shp.tile([128, blocks * 128], bf16)

            # ---- phase 1: dense transform + g1 shard + exchange ----
            for b in range(blocks):
                pt = pph1.tile([128, chid], f32, tag="ph1")
                nc.tensor.matmul(pt[:], x_sb[:, b * 128:(b + 1) * 128],
                                 w1_sb[:], start=True, stop=True)
                nc.vector.tensor_scalar(
                    g1s[:, b * chid:(b + 1) * chid], pt[:],
                    dis_pm[:, b:b + 1], None, mult)
            def exchange(bounce, tabs, j):
                if DBG_NO_COLL:
                    nc.sync.dma_start(tabs[j][0:qrows, :], bounce[j][:])
                else:
                    nc.gpsimd.collective_compute(
                        "AllGather", mybir.AluOpType.bypass,
                        replica_groups=[list(range(NCORES))],
                        ins=[bounce[j].opt()], outs=[tabs[j].opt()])

            g1s3 = g1s[:].rearrange("p (b c) -> p b c", c=chid)
            for j in range(PIECES):
                nc.sync.dma_start(
                    bounce1[j][:].rearrange("(b p) c -> p b c", p=128),
                    g1s3[:, j * bpp:(j + 1) * bpp, :])
                exchange(bounce1, tab1, j)

            # ---- gather/aggregate layers ----
            def aggregate(layer, tabs, gself):
                bias_sb = b1_sb if layer == 1 else b2_sb
                tile_cursor = 0
                batch_i = 0
                b0 = 0
                for g in supers:
                    assert g % 4 == 0
                    nbank = g // 4
                    psums = [pbig.tile([128, 512], f32, name="acc",
                                       tag="acc") for _ in range(nbank)]

                    def pacc(bi):
                        return psums[bi // 4][:, (bi % 4) * 128:
                                              (bi % 4) * 128 + 128]

                    # program-order matmul sequence; find last item per bank
                    seq = [("self", bi) for bi in range(g)]
                    if not DBG_NO_GATHER:
                        for qq in range(PIECES):
                            for bi in range(g):
                                nt = int(tiles_bq[(b0 + bi) * PIECES + qq])
                                for t in range(nt):
                                    seq.append(("edge", qq, bi, t))
                    last_per_bank = {}
                    for item in seq:
                        bi = item[1] if item[0] == "self" else item[2]
                        last_per_bank[bi // 4] = item

                    # seeds: one block-diagonal K=4 matmul per bank
                    for k in range(nbank):
                        gb = (b0 + k * 4) // 4  # global bank index
                        nc.tensor.matmul(
                            psums[k][:],
                            invd_b4[:, gb * 128:(gb + 1) * 128],
                            bias_sb[:], start=True, stop=False)
                    # self loops
                    for bi in range(g):
                        b = b0 + bi
                        nc.tensor.matmul(
                            pacc(bi), eye_sb[:],
                            gself[:, b * 128:(b + 1) * 128],
                            start=False,
                            stop=(last_per_bank[bi // 4] == ("self", bi)))
                    # edge tiles, batched per source piece
                    for qq in range(PIECES):
                        nb = batches[batch_i]
                        batch_i += 1
                        if nb == 0 or DBG_NO_GATHER:
                            tile_cursor += nb
                            continue
                        st = stp.tile([128, nb, 128], bf16, tag="stage")
                        nc.gpsimd.dma_gather(
                            st[:], tabs[qq][:],
                            idxp_sb[:, tile_cursor * 8:
                                    (tile_cursor + nb) * 8],
                            nb * 128, nb * 128, 128,
                            single_packet=False)
                        t_local = 0
                        for bi in range(g):
                            b = b0 + bi
                            nt = int(tiles_bq[b * PIECES + qq])
                            for t in range(nt):
                                gcol = tile_cursor + t_local
                                mk = mp.tile([128, 128], bf16, tag="mask")
                                nc.vector.tensor_scalar(
                                    mk[:], iota_sb[:],
                                    relp_sb[:, gcol:gcol + 1], None, iseq)
                                stop = (last_per_bank[bi // 4] ==
                                        ("edge", qq, bi, t))
                                nc.tensor.matmul(
                                    pacc(bi), mk[:],
                                    st[:, t_local:t_local + 1, :].squeeze(),
                                    start=False, stop=stop)
                                t_local += 1
                        tile_cursor += nb
                    # evictions
                    for bi in range(g):
                        b = b0 + bi
                        if layer == 1:
                            h1r = wp.tile([128, chid], bf16, tag="h1r")
                            nc.vector.tensor_scalar(
                                h1r[:], pacc(bi), dis_pm[:, b:b + 1], 0.0,
                                mult, amax)
                            ptr = pptr.tile([128, chid], bf16, tag="ptr")
                            nc.tensor.transpose(ptr[:], h1r[:], eye_sb[:])
                            ht = wp.tile([128, chid], bf16, tag="ht")
                            nc.vector.tensor_copy(ht[:], ptr[:])
                            pg = ppg.tile([128, 128], f32, tag="pg")
                            nc.tensor.matmul(pg[:], ht[:], w2_sb[:],
                                             start=True, stop=True)
                            nc.vector.tensor_scalar(
                                g2s[:, b * 128:(b + 1) * 128], pg[:],
                                dis_pm[:, b:b + 1], None, mult)
                        else:
                            ob = op_.tile([128, cout], f32, tag="ob")
                            nc.vector.tensor_scalar(
                                ob[:], pacc(bi)[:, :cout],
                                dis_pm[:, b:b + 1], None, mult)
                            nc.sync.dma_start(
                                out_t[b * 128:(b + 1) * 128, :], ob[:])
                    b0 += g
                if layer == 1:
                    g2s3 = g2s[:].rearrange("p (b c) -> p b c", c=128)
                    for j in range(PIECES):
                        nc.sync.dma_start(
                            bounce2[j][:].rearrange("(b p) c -> p b c",
                                                    p=128),
                            g2s3[:, j * bpp:(j + 1) * bpp, :])
                        exchange(bounce2, tab2, j)

            aggregate(1, tab1, g1s)
            aggregate(2, tab2, g2s)

    nc.compile()
    return nc


def run_config(inputs, cfg, run=None):
    from concourse.bass_utils import run_bass_kernel_spmd

    x = np.asarray(inputs["x"], np.float32)
    edge_index = np.asarray(inputs["edge_index"])
    meta, in_maps = _host_inputs(
        x, edge_index, inputs["W1"], inputs["b1"], inputs["W2"],
        inputs["b2"], cfg)
    nc = _build_program(cfg, meta)
    if run is None:
        def run(nc, in_maps):
            return run_bass_kernel_spmd(
                nc, in_maps, list(range(NCORES))).results
    results = run(nc, in_maps)
    bucket = _derive(cfg)["bucket"]
    out = np.concatenate(
        [results[c]["out"][:bucket] for c in range(NCORES)], axis=0)
    return np.ascontiguousarray(out.astype(np.float32))


def kernel(**inputs):
    return run_config(inputs, CFG_FULL)



# revision 4
# speedup vs baseline: 2.1852x; 2.1852x over previous
"""Two-layer GCN (GCNConv x2, PyG-style symmetric normalization) on 8 trn2
NeuronCores.

Vertex-cut graph parallelism, v2:
  - Nodes are assigned to (core, block, slot) positions by a host-side
    LPT balancer so per-(block, piece) edge counts are nearly uniform
    across cores (the SPMD tile schedule is shared by all cores, so the
    max over cores determines the padded tile count).
  - Normalization is factored:  out[d] = b + dis[d]*(sum_{e:col=d}
    g[row_e] + g[d]),  g[n] = dis[n]*(x@W)[n],  dis = 1/sqrt(deg).
    Aggregation is a pure indicator matmul over 128-edge tiles.
  - Layer-1 accumulates transposed ([ch, dst] in PSUM) so the relu
    eviction needs no PE transpose: relu is done unscaled on the Scalar
    engine (relu(dis*x) = dis*relu(x), dis>0) and the dis^2 factor is
    folded into the post-W2 scale.
  - One-hot masks are built in one wide DVE tensor_tensor per gather
    batch using stride-0 broadcast APs (iota == rel), instead of one
    tensor_scalar per tile.
  - Gathers run on 4 SWDGE queues (one per table piece) so batches
    drain concurrently on the DMA engines.
  - All PSUM evictions run on the Scalar (ACT) engine; DVE only builds
    masks; GpSimd only generates gather descriptors and triggers
    collectives.
  - Layer-2 table pieces are exchanged as soon as their block range has
    been evicted, overlapping the AllGathers with layer-1 aggregation.
"""

import math

import numpy as np

try:
    from ml_dtypes import bfloat16 as np_bf16
except ImportError:  # pragma: no cover
    np_bf16 = None

CFG_FULL = dict(N=100000, E=1600000, CIN=128, CHID=128, COUT=64)

NCORES = 8
PIECES = 4  # table pieces / AllGather splits (int16 gather index limit)
SUPER = 8  # dst blocks per gather batch group


def _derive(cfg):
    n = cfg["N"]
    bucket = n // NCORES
    assert bucket * NCORES == n
    blocks = math.ceil(bucket / 128)
    blocks = math.ceil(blocks / (4 * PIECES)) * (4 * PIECES)
    shard = blocks * 128
    bpp = blocks // PIECES  # blocks per piece
    qrows = shard // PIECES  # rows per piece per core
    chunk = qrows * NCORES  # rows of one assembled table piece
    assert chunk <= 32600, chunk  # int16 gather index limit
    supers = [SUPER] * (blocks // SUPER)
    if blocks % SUPER:
        supers.append(blocks % SUPER)
    return dict(bucket=bucket, blocks=blocks, shard=shard, qrows=qrows,
                chunk=chunk, supers=supers, bpp=bpp)


def _assign_nodes(edge_index, cfg):
    """LPT-balance destination load: node -> (core, block, slot-in-block).

    Returns slot_of_node [N] (global slot id in 0..NCORES*shard) and
    node_of_slot [NCORES*shard] (-1 for padding slots).
    """
    d = _derive(cfg)
    n, blocks, shard = cfg["N"], d["blocks"], d["shard"]
    nbins = NCORES * blocks
    deg_in = np.bincount(edge_index[1], minlength=n).astype(np.int64)
    order = np.argsort(-deg_in, kind="stable")
    cap = math.ceil(n / nbins)
    assert cap <= 128
    load = np.zeros(nbins, np.int64)
    slot_of_node = np.empty(n, np.int64)
    fill = np.zeros(nbins, np.int64)
    for r in range(cap):
        chunk_nodes = order[r * nbins:(r + 1) * nbins]
        if chunk_nodes.size == 0:
            break
        bins = np.argsort(load, kind="stable")[:chunk_nodes.size]
        load[bins] += deg_in[chunk_nodes]
        core = bins // blocks
        blk = bins % blocks
        slot_of_node[chunk_nodes] = core * shard + blk * 128 + fill[bins]
        fill[bins] += 1
    node_of_slot = np.full(NCORES * shard, -1, np.int64)
    node_of_slot[slot_of_node] = np.arange(n)
    return slot_of_node, node_of_slot


def _preprocess(edge_index, slot_of_node, cfg):
    """Bucket & sort edges (by destination core/block/piece), build
    per-core gather/mask planes with the shared tile schedule."""
    d = _derive(cfg)
    blocks, qrows, shard = d["blocks"], d["qrows"], d["shard"]
    src = slot_of_node[edge_index[0].astype(np.int64)]
    dst = slot_of_node[edge_index[1].astype(np.int64)]

    c_dst = dst // shard
    d_l = dst - c_dst * shard
    blk = d_l // 128
    rel = (d_l % 128).astype(np.float32)
    c_src = src // shard
    r_l = src - c_src * shard
    q = r_l // qrows
    ric = (c_src * qrows + r_l % qrows).astype(np.int64)  # row in chunk q

    nbq = blocks * PIECES
    key_bq = blk * PIECES + q
    counts = np.zeros((NCORES, nbq), np.int64)
    for c in range(NCORES):
        m = c_dst == c
        counts[c] = np.bincount(key_bq[m], minlength=nbq)
    tiles_bq = np.ceil(counts.max(axis=0) / 128).astype(np.int64)  # [nbq]

    # tile schedule in program order: (super, piece, block in super, tile)
    order_bq = []
    supers = d["supers"]
    b0 = 0
    batches = []  # tiles per (super, piece) gather batch
    for g in supers:
        for qq in range(PIECES):
            nt = 0
            for b in range(b0, b0 + g):
                order_bq.append((b, qq))
                nt += int(tiles_bq[b * PIECES + qq])
            batches.append(nt)
        b0 += g
    tot_tiles = int(tiles_bq.sum())
    assert sum(batches) == tot_tiles and tot_tiles > 0

    off_bq = np.zeros(nbq, np.int64)
    acc = 0
    for (b, qq) in order_bq:
        off_bq[b * PIECES + qq] = acc
        acc += int(tiles_bq[b * PIECES + qq])

    per_core = []
    for c in range(NCORES):
        m = c_dst == c
        okey = (blk[m] * PIECES + q[m]).astype(np.int64)
        sort = np.argsort(okey, kind="stable")
        okey_s = okey[sort]
        e_rel = rel[m][sort]
        e_ric = ric[m][sort]
        slot_base = off_bq[okey_s] * 128
        grp_start = np.searchsorted(okey_s, okey_s)
        within = np.arange(okey_s.size) - grp_start
        slots = slot_base + within
        idx_flat = np.zeros(tot_tiles * 128, np.int16)
        rel_flat = np.full(tot_tiles * 128, -1.0, np.float32)
        idx_flat[slots] = e_ric.astype(np.int16)
        rel_flat[slots] = e_rel
        idx16 = idx_flat.reshape(tot_tiles * 8, 16).T  # [16, tiles*8]
        idx_plane = np.tile(idx16, (8, 1)).copy()
        rel_plane = np.ascontiguousarray(
            rel_flat.reshape(tot_tiles, 128).T)  # [128, tot_tiles]
        per_core.append(dict(idx_plane=idx_plane, rel_plane=rel_plane))

    meta = dict(d=d, tiles_bq=tiles_bq, batches=batches, tot_tiles=tot_tiles,
                supers=supers)
    return meta, per_core


def _bf16(a):
    a = np.asarray(a, np.float32)
    if np_bf16 is not None:
        return a.astype(np_bf16)
    return a  # fall back: ship f32 (kernel would need dtype change)


def _host_inputs(x, edge_index, W1, b1, W2, b2, cfg):
    d = _derive(cfg)
    blocks, shard = d["blocks"], d["shard"]
    n, cin = cfg["N"], cfg["CIN"]
    chid, cout = cfg["CHID"], cfg["COUT"]
    slot_of_node, node_of_slot = _assign_nodes(edge_index, cfg)
    meta, per_core = _preprocess(edge_index, slot_of_node, cfg)
    meta["node_of_slot"] = node_of_slot

    col = edge_index[1].astype(np.int64)
    deg = (np.bincount(col, minlength=n) + 1).astype(np.float32)

    w1 = _bf16(W1)
    w2p = np.zeros((chid, 128), np.float32)
    w2p[:, :cout] = np.asarray(W2, np.float32)
    w2p = _bf16(w2p)
    b1r = _bf16(np.asarray(b1, np.float32).reshape(1, chid))
    b2d = np.zeros((4, 512), np.float32)
    for k in range(4):
        b2d[k, k * 128:k * 128 + cout] = np.asarray(b2, np.float32)
    iota = _bf16(np.broadcast_to(
        np.arange(128, dtype=np.float32)[None, :], (128, 128)))
    eye = _bf16(np.eye(128, dtype=np.float32))

    x_np = np.asarray(x, np.float32)
    in_maps = []
    for c in range(NCORES):
        slots = node_of_slot[c * shard:(c + 1) * shard]
        valid = slots >= 0
        xs = np.zeros((shard, cin), np.float32)
        xs[valid] = x_np[slots[valid]]
        x_ct = _bf16(np.ascontiguousarray(xs.T))  # [cin, shard] bf16
        degs = np.ones(shard, np.float32)
        degs[valid] = deg[slots[valid]]
        invd = np.sqrt(degs)
        dis = 1.0 / invd
        dis_pm = np.ascontiguousarray(dis.reshape(blocks, 128).T)
        dis2_pm = np.ascontiguousarray((dis * dis).reshape(blocks, 128).T)
        # [4, (blocks//4)*128]: [k, g*128+p] = invd[(4g+k)*128+p]
        invd_b4 = np.ascontiguousarray(
            invd.reshape(blocks // 4, 4, 128).transpose(1, 0, 2)
            .reshape(4, -1))
        invd_row = _bf16(invd.reshape(1, shard))
        in_maps.append({
            "x_ct": x_ct, "dis_pm": dis_pm, "dis2_pm": dis2_pm,
            "invd_b4": invd_b4, "invd_row": invd_row,
            "idx_plane": per_core[c]["idx_plane"],
            "rel_plane": _bf16(per_core[c]["rel_plane"]),
            "w1": w1, "w2p": w2p, "b1r": b1r, "b2d": b2d,
            "iota": iota, "eye": eye,
        })
    return meta, in_maps


def _build_program(cfg, meta):
    import concourse.bacc as bacc
    import concourse.mybir as mybir
    from concourse import tile

    d = meta["d"]
    blocks, shard, qrows, chunk, bpp = (d["blocks"], d["shard"], d["qrows"],
                                        d["chunk"], d["bpp"])
    supers = meta["supers"]
    tiles_bq = meta["tiles_bq"]
    tot_tiles = meta["tot_tiles"]
    batches = meta["batches"]
    cin, chid, cout = cfg["CIN"], cfg["CHID"], cfg["COUT"]

    bf16 = mybir.dt.bfloat16
    f32 = mybir.dt.float32
    i16 = mybir.dt.int16
    mult = mybir.AluOpType.mult
    iseq = mybir.AluOpType.is_equal
    Relu = mybir.ActivationFunctionType.Relu
    Copy = mybir.ActivationFunctionType.Copy

    nc = bacc.Bacc("TRN2", target_bir_lowering=False, debug=False,
                   num_devices=NCORES, num_swdge_queues=4)

    x_ct = nc.dram_tensor("x_ct", [cin, shard], bf16, kind="ExternalInput")
    dis_pm_t = nc.dram_tensor("dis_pm", [128, blocks], f32,
                              kind="ExternalInput")
    dis2_pm_t = nc.dram_tensor("dis2_pm", [128, blocks], f32,
                               kind="ExternalInput")
    invd_b4_t = nc.dram_tensor("invd_b4", [4, (blocks // 4) * 128], f32,
                               kind="ExternalInput")
    invd_row_t = nc.dram_tensor("invd_row", [1, shard], bf16,
                                kind="ExternalInput")
    idxp_t = nc.dram_tensor("idx_plane", [128, tot_tiles * 8], i16,
                            kind="ExternalInput")
    relp_t = nc.dram_tensor("rel_plane", [128, tot_tiles], bf16,
                            kind="ExternalInput")
    w1_t = nc.dram_tensor("w1", [cin, chid], bf16, kind="ExternalInput")
    w2p_t = nc.dram_tensor("w2p", [chid, 128], bf16, kind="ExternalInput")
    b1r_t = nc.dram_tensor("b1r", [1, chid], bf16, kind="ExternalInput")
    b2d_t = nc.dram_tensor("b2d", [4, 512], f32, kind="ExternalInput")
    iota_t = nc.dram_tensor("iota", [128, 128], bf16, kind="ExternalInput")
    eye_t = nc.dram_tensor("eye", [128, 128], bf16, kind="ExternalInput")
    out_t = nc.dram_tensor("out", [shard, cout], f32, kind="ExternalOutput")

    # Shared-scratchpad AllGather outputs (faster HBM-HBM collectives)
    tab1 = [nc.dram_tensor(f"t1_{j}", [chunk, chid], bf16,
                           addr_space="Shared") for j in range(PIECES)]
    tab2 = [nc.dram_tensor(f"t2_{j}", [chunk, 128], bf16,
                           addr_space="Shared") for j in range(PIECES)]

    with tile.TileContext(nc) as tc:
        with (
            tc.tile_pool(name="dram", bufs=1, space="DRAM") as dram,
            tc.tile_pool(name="const", bufs=1) as cp,
            tc.tile_pool(name="shards", bufs=1) as shp,
            tc.tile_pool(name="xs", bufs=4) as xp,
            tc.tile_pool(name="stage", bufs=3) as stp,
            tc.tile_pool(name="idxs", bufs=6) as ixp,
            tc.tile_pool(name="masks", bufs=2) as mp,
            tc.tile_pool(name="h1t", bufs=3) as hp,
            tc.tile_pool(name="outp", bufs=4) as op_,
            tc.tile_pool(name="pbig", bufs=4, space="PSUM") as pbig,
            tc.tile_pool(name="pph1", bufs=2, space="PSUM") as pph1,
            tc.tile_pool(name="ppg", bufs=2, space="PSUM") as ppg,
        ):
            # ---- DRAM scratch (collective inputs must be Local) ----
            bounce1 = [dram.tile([qrows, chid], bf16, name=f"bo1_{j}",
                                 tag=f"bo1_{j}") for j in range(PIECES)]
            bounce2 = [dram.tile([qrows, 128], bf16, name=f"bo2_{j}",
                                 tag=f"bo2_{j}") for j in range(PIECES)]

            # ---- constants ----
            iota_sb = cp.tile([128, 128], bf16)
            nc.sync.dma_start(iota_sb[:], iota_t[:])
            eye_sb = cp.tile([128, 128], bf16)
            nc.sync.dma_start(eye_sb[:], eye_t[:])
            w1_sb = cp.tile([cin, chid], bf16)
            nc.sync.dma_start(w1_sb[:], w1_t[:])
            w2_sb = cp.tile([chid, 128], bf16)
            nc.sync.dma_start(w2_sb[:], w2p_t[:])
            b1_sb = cp.tile([1, chid], bf16)
            nc.sync.dma_start(b1_sb[:], b1r_t[:])
            b2_sb = cp.tile([4, 512], f32)
            nc.sync.dma_start(b2_sb[:], b2d_t[:])
            relp_sb = cp.tile([128, tot_tiles], bf16)
            nc.sync.dma_start(relp_sb[:], relp_t[:])

            dis_pm = cp.tile([128, blocks], f32)
            nc.sync.dma_start(dis_pm[:], dis_pm_t[:])
            dis2_pm = cp.tile([128, blocks], f32)
            nc.sync.dma_start(dis2_pm[:], dis2_pm_t[:])
            invd_b4 = cp.tile([4, (blocks // 4) * 128], f32)
            nc.sync.dma_start(invd_b4[:], invd_b4_t[:])
            invd_row = cp.tile([1, shard], bf16)
            nc.sync.dma_start(invd_row[:], invd_row_t[:])

            g1s = shp.tile([128, blocks * chid], bf16)
            g2s = shp.tile([128, blocks * 128], bf16)

            def exchange(bounce, tabs, j):
                nc.gpsimd.collective_compute(
                    "AllGather", mybir.AluOpType.bypass,
                    replica_groups=[list(range(NCORES))],
                    ins=[bounce[j].opt()], outs=[tabs[j][:].opt()])

            # ---- phase 1: dense transform -> g1 shard, exchange per piece
            for j in range(PIECES):
                for b in range(j * bpp, (j + 1) * bpp):
                    xb = xp.tile([cin, 128], bf16, tag="xb")
                    nc.sync.dma_start(xb[:], x_ct[:, b * 128:(b + 1) * 128])
                    pt = pph1.tile([128, chid], f32, tag="ph1")
                    nc.tensor.matmul(pt[:], xb[:], w1_sb[:],
                                     start=True, stop=True)
                    nc.scalar.activation(
                        g1s[:, b * chid:(b + 1) * chid], pt[:], Copy,
                        bias=0.0, scale=dis_pm[:, b:b + 1])
                g1s3 = g1s[:].rearrange("p (b c) -> p b c", c=chid)
                nc.sync.dma_start(
                    bounce1[j][:].rearrange("(b p) c -> p b c", p=128),
                    g1s3[:, j * bpp:(j + 1) * bpp, :])
                exchange(bounce1, tab1, j)

            # ---- gather/aggregate layers ----
            # layer 1: psum [ch, dst]  (lhsT=st, rhs=mask)
            # layer 2: psum [dst, ch]  (lhsT=mask, rhs=st)
            l2x_done = [False] * PIECES

            def l2_exchange_ready(b_done):
                """Fire layer-2 exchanges whose block range is evicted."""
                for j in range(PIECES):
                    if not l2x_done[j] and b_done >= (j + 1) * bpp:
                        g2s3 = g2s[:].rearrange("p (b c) -> p b c", c=128)
                        nc.sync.dma_start(
                            bounce2[j][:].rearrange("(b p) c -> p b c",
                                                    p=128),
                            g2s3[:, j * bpp:(j + 1) * bpp, :])
                        exchange(bounce2, tab2, j)
                        l2x_done[j] = True

            def aggregate(layer, tabs):
                tile_cursor = 0
                batch_i = 0
                b0 = 0
                for g in supers:
                    assert g % 4 == 0
                    nbank = g // 4
                    psums = [pbig.tile([128, 512], f32, name="acc",
                                       tag="acc") for _ in range(nbank)]

                    def pacc(bi):
                        return psums[bi // 4][:, (bi % 4) * 128:
                                              (bi % 4) * 128 + 128]

                    # program-order matmul sequence; find last item per bank
                    seq = [("self", bi) for bi in range(g)]
                    for qq in range(PIECES):
                        for bi in range(g):
                            nt = int(tiles_bq[(b0 + bi) * PIECES + qq])
                            for t in range(nt):
                                seq.append(("edge", qq, bi, t))
                    last_per_bank = {}
                    for item in seq:
                        bi = item[1] if item[0] == "self" else item[2]
                        last_per_bank[bi // 4] = item

                    # seeds
                    for k in range(nbank):
                        gb = (b0 + k * 4) // 4  # global bank index
                        if layer == 1:
                            # psum[ch, dst] = b1[ch] * invd[dst]
                            nc.tensor.matmul(
                                psums[k][:], b1_sb[:],
                                invd_row[:, (b0 + k * 4) * 128:
                                         (b0 + k * 4) * 128 + 512],
                                start=True, stop=False)
                        else:
                            # psum[dst, ch4] = invd[dst] * b2 blockdiag
                            nc.tensor.matmul(
                                psums[k][:],
                                invd_b4[:, gb * 128:(gb + 1) * 128],
                                b2_sb[:], start=True, stop=False)
                    # self loops
                    for bi in range(g):
                        b = b0 + bi
                        stop = last_per_bank[bi // 4] == ("self", bi)
                        if layer == 1:
                            nc.tensor.matmul(
                                pacc(bi), g1s[:, b * chid:(b + 1) * chid],
                                eye_sb[:], start=False, stop=stop)
                        else:
                            nc.tensor.matmul(
                                pacc(bi), eye_sb[:],
                                g2s[:, b * 128:(b + 1) * 128],
                                start=False, stop=stop)
                    # edge tiles, batched per source piece
                    for qq in range(PIECES):
                        nb = batches[batch_i]
                        batch_i += 1
                        if nb == 0:
                            continue
                        idxb = ixp.tile([128, nb * 8], i16, tag="idxb")
                        nc.sync.dma_start(
                            idxb[:], idxp_t[:, tile_cursor * 8:
                                            (tile_cursor + nb) * 8])
                        st = stp.tile([128, nb, 128], bf16, tag="stage")
                        nc.gpsimd.dma_gather(
                            st[:], tabs[qq][:], idxb[:],
                            nb * 128, nb * 128, 128,
                            single_packet=False, queue_num=qq % 4)
                        # one wide mask build for the whole batch
                        mk = mp.tile([128, nb, 128], bf16, tag="mask")
                        iota_b = iota_sb[:].rearrange(
                            "p (t c) -> p t c", t=1).broadcast_to(
                                [128, nb, 128])
                        rel_b = relp_sb[:, tile_cursor:
                                        tile_cursor + nb].rearrange(
                            "p (t o) -> p t o", o=1).broadcast_to(
                                [128, nb, 128])
                        nc.vector.tensor_tensor(mk[:], iota_b, rel_b, iseq)
                        t_local = 0
                        for bi in range(g):
                            b = b0 + bi
                            nt = int(tiles_bq[b * PIECES + qq])
                            for t in range(nt):
                                stop = (last_per_bank[bi // 4] ==
                                        ("edge", qq, bi, t))
                                st_t = st[:, t_local, :].squeeze()
                                mk_t = mk[:, t_local, :].squeeze()
                                if layer == 1:
                                    nc.tensor.matmul(pacc(bi), st_t, mk_t,
                                                     start=False, stop=stop)
                                else:
                                    nc.tensor.matmul(pacc(bi), mk_t, st_t,
                                                     start=False, stop=stop)
                                t_local += 1
                        tile_cursor += nb
                    # evictions
                    if layer == 1:
                        for k in range(nbank):
                            h1b = hp.tile([128, 512], bf16, tag="h1b")
                            nc.scalar.activation(h1b[:], psums[k][:], Relu)
                            for kk in range(4):
                                bi = k * 4 + kk
                                b = b0 + bi
                                pg = ppg.tile([128, 128], f32, tag="pg")
                                nc.tensor.matmul(
                                    pg[:], h1b[:, kk * 128:(kk + 1) * 128],
                                    w2_sb[:], start=True, stop=True)
                                nc.scalar.activation(
                                    g2s[:, b * 128:(b + 1) * 128], pg[:],
                                    Copy, bias=0.0,
                                    scale=dis2_pm[:, b:b + 1])
                        l2_exchange_ready(b0 + g)
                    else:
                        for bi in range(g):
                            b = b0 + bi
                            ob = op_.tile([128, cout], f32, tag="ob")
                            nc.scalar.activation(
                                ob[:], pacc(bi)[:, :cout], Copy, bias=0.0,
                                scale=dis_pm[:, b:b + 1])
                            nc.sync.dma_start(
                                out_t[b * 128:(b + 1) * 128, :], ob[:])
                    b0 += g

            aggregate(1, tab1)
            aggregate(2, tab2)

    nc.compile()
    return nc


def run_config(inputs, cfg, run=None):
    from concourse.bass_utils import run_bass_kernel_spmd

    x = np.asarray(inputs["x"], np.float32)
    edge_index = np.asarray(inputs["edge_index"])
    meta, in_maps = _host_inputs(
        x, edge_index, inputs["W1"], inputs["b1"], inputs["W2"],
        inputs["b2"], cfg)
    nc = _build_program(cfg, meta)
    if run is None:
        def run(nc, in_maps):
            return run_bass_kernel_spmd(
                nc, in_maps, list(range(NCORES))).results
    results = run(nc, in_maps)
    d = _derive(cfg)
    shard = d["shard"]
    all_rows = np.concatenate(
        [results[c]["out"] for c in range(NCORES)], axis=0)
    node_of_slot = meta["node_of_slot"]
    valid = node_of_slot >= 0
    out = np.empty((cfg["N"], cfg["COUT"]), np.float32)
    out[node_of_slot[valid]] = all_rows[valid]
    return np.ascontiguousarray(out)


def kernel(**inputs):
    return run_config(inputs, CFG_FULL)


# revision 5
# speedup vs baseline: 2.6268x; 1.2021x over previous
"""Two-layer GCN (GCNConv x2, PyG-style symmetric normalization) on 8 trn2
NeuronCores.

Vertex-cut graph parallelism, v2:
  - Nodes are assigned to (core, block, slot) positions by a host-side
    LPT balancer so per-(block, piece) edge counts are nearly uniform
    across cores (the SPMD tile schedule is shared by all cores, so the
    max over cores determines the padded tile count).
  - Normalization is factored:  out[d] = b + dis[d]*(sum_{e:col=d}
    g[row_e] + g[d]),  g[n] = dis[n]*(x@W)[n],  dis = 1/sqrt(deg).
    Aggregation is a pure indicator matmul over 128-edge tiles.
  - Layer-1 accumulates transposed ([ch, dst] in PSUM) so the relu
    eviction needs no PE transpose: relu is done unscaled on the Scalar
    engine (relu(dis*x) = dis*relu(x), dis>0) and the dis^2 factor is
    folded into the post-W2 scale.
  - One-hot masks are built in one wide DVE tensor_tensor per gather
    batch using stride-0 broadcast APs (iota == rel), instead of one
    tensor_scalar per tile.
  - Gathers run on 4 SWDGE queues (one per table piece) so batches
    drain concurrently on the DMA engines.
  - All PSUM evictions run on the Scalar (ACT) engine; DVE only builds
    masks; GpSimd only generates gather descriptors and triggers
    collectives.
  - Layer-2 table pieces are exchanged as soon as their block range has
    been evicted, overlapping the AllGathers with layer-1 aggregation.
"""

import math

import numpy as np

try:
    from ml_dtypes import bfloat16 as np_bf16
except ImportError:  # pragma: no cover
    np_bf16 = None

CFG_FULL = dict(N=100000, E=1600000, CIN=128, CHID=128, COUT=64)

NCORES = 8
PIECES = 4  # table pieces / AllGather splits (int16 gather index limit)
SUPER = 8  # dst blocks per gather batch group


def _derive(cfg):
    n = cfg["N"]
    bucket = n // NCORES
    assert bucket * NCORES == n
    blocks = math.ceil(bucket / 128)
    blocks = math.ceil(blocks / (4 * PIECES)) * (4 * PIECES)
    shard = blocks * 128
    bpp = blocks // PIECES  # blocks per piece
    qrows = shard // PIECES  # rows per piece per core
    chunk = qrows * NCORES  # rows of one assembled table piece
    assert chunk <= 32600, chunk  # int16 gather index limit
    supers = [SUPER] * (blocks // SUPER)
    if blocks % SUPER:
        supers.append(blocks % SUPER)
    return dict(bucket=bucket, blocks=blocks, shard=shard, qrows=qrows,
                chunk=chunk, supers=supers, bpp=bpp)


def _assign_nodes(edge_index, cfg):
    """LPT-balance destination load: node -> (core, block, slot-in-block).

    Returns slot_of_node [N] (global slot id in 0..NCORES*shard) and
    node_of_slot [NCORES*shard] (-1 for padding slots).
    """
    d = _derive(cfg)
    n, blocks, shard = cfg["N"], d["blocks"], d["shard"]
    nbins = NCORES * blocks
    deg_in = np.bincount(edge_index[1], minlength=n).astype(np.int64)
    order = np.argsort(-deg_in, kind="stable")
    cap = math.ceil(n / nbins)
    assert cap <= 128
    load = np.zeros(nbins, np.int64)
    slot_of_node = np.empty(n, np.int64)
    fill = np.zeros(nbins, np.int64)
    for r in range(cap):
        chunk_nodes = order[r * nbins:(r + 1) * nbins]
        if chunk_nodes.size == 0:
            break
        bins = np.argsort(load, kind="stable")[:chunk_nodes.size]
        load[bins] += deg_in[chunk_nodes]
        core = bins // blocks
        blk = bins % blocks
        slot_of_node[chunk_nodes] = core * shard + blk * 128 + fill[bins]
        fill[bins] += 1
    node_of_slot = np.full(NCORES * shard, -1, np.int64)
    node_of_slot[slot_of_node] = np.arange(n)
    return slot_of_node, node_of_slot


def _preprocess(edge_index, slot_of_node, cfg):
    """Bucket & sort edges (by destination core/block/piece), build
    per-core gather/mask planes with the shared tile schedule."""
    d = _derive(cfg)
    blocks, qrows, shard = d["blocks"], d["qrows"], d["shard"]
    src = slot_of_node[edge_index[0].astype(np.int64)]
    dst = slot_of_node[edge_index[1].astype(np.int64)]

    c_dst = dst // shard
    d_l = dst - c_dst * shard
    blk = d_l // 128
    rel = (d_l % 128).astype(np.float32)
    c_src = src // shard
    r_l = src - c_src * shard
    q = r_l // qrows
    ric = (c_src * qrows + r_l % qrows).astype(np.int64)  # row in chunk q

    nbq = blocks * PIECES
    key_bq = blk * PIECES + q
    counts = np.zeros((NCORES, nbq), np.int64)
    for c in range(NCORES):
        m = c_dst == c
        counts[c] = np.bincount(key_bq[m], minlength=nbq)
    tiles_bq = np.ceil(counts.max(axis=0) / 128).astype(np.int64)  # [nbq]

    # tile schedule in program order: (super, piece, block in super, tile)
    order_bq = []
    supers = d["supers"]
    b0 = 0
    batches = []  # tiles per (super, piece) gather batch
    for g in supers:
        for qq in range(PIECES):
            nt = 0
            for b in range(b0, b0 + g):
                order_bq.append((b, qq))
                nt += int(tiles_bq[b * PIECES + qq])
            batches.append(nt)
        b0 += g
    tot_tiles = int(tiles_bq.sum())
    assert sum(batches) == tot_tiles and tot_tiles > 0

    off_bq = np.zeros(nbq, np.int64)
    acc = 0
    for (b, qq) in order_bq:
        off_bq[b * PIECES + qq] = acc
        acc += int(tiles_bq[b * PIECES + qq])

    per_core = []
    for c in range(NCORES):
        m = c_dst == c
        okey = (blk[m] * PIECES + q[m]).astype(np.int64)
        sort = np.argsort(okey, kind="stable")
        okey_s = okey[sort]
        e_rel = rel[m][sort]
        e_ric = ric[m][sort]
        slot_base = off_bq[okey_s] * 128
        grp_start = np.searchsorted(okey_s, okey_s)
        within = np.arange(okey_s.size) - grp_start
        slots = slot_base + within
        idx_flat = np.zeros(tot_tiles * 128, np.int16)
        rel_flat = np.full(tot_tiles * 128, -1.0, np.float32)
        idx_flat[slots] = e_ric.astype(np.int16)
        rel_flat[slots] = e_rel
        idx16 = idx_flat.reshape(tot_tiles * 8, 16).T  # [16, tiles*8]
        idx_plane = np.tile(idx16, (8, 1)).copy()
        rel_plane = np.ascontiguousarray(
            rel_flat.reshape(tot_tiles, 128).T)  # [128, tot_tiles]
        per_core.append(dict(idx_plane=idx_plane, rel_plane=rel_plane))

    meta = dict(d=d, tiles_bq=tiles_bq, batches=batches, tot_tiles=tot_tiles,
                supers=supers)
    return meta, per_core


def _bf16(a):
    a = np.asarray(a, np.float32)
    if np_bf16 is not None:
        return a.astype(np_bf16)
    return a  # fall back: ship f32 (kernel would need dtype change)


def _host_inputs(x, edge_index, W1, b1, W2, b2, cfg):
    d = _derive(cfg)
    blocks, shard = d["blocks"], d["shard"]
    n, cin = cfg["N"], cfg["CIN"]
    chid, cout = cfg["CHID"], cfg["COUT"]
    slot_of_node, node_of_slot = _assign_nodes(edge_index, cfg)
    meta, per_core = _preprocess(edge_index, slot_of_node, cfg)
    meta["node_of_slot"] = node_of_slot

    col = edge_index[1].astype(np.int64)
    deg = (np.bincount(col, minlength=n) + 1).astype(np.float32)

    w1 = _bf16(W1)
    w2p = np.zeros((chid, 128), np.float32)
    w2p[:, :cout] = np.asarray(W2, np.float32)
    w2p = _bf16(w2p)
    b1r = _bf16(np.asarray(b1, np.float32).reshape(1, chid))
    ones_row = _bf16(np.ones((1, 128), np.float32))
    b2d = np.zeros((4, 512), np.float32)
    for k in range(4):
        b2d[k, k * 128:k * 128 + cout] = np.asarray(b2, np.float32)
    iota = _bf16(np.broadcast_to(
        np.arange(128, dtype=np.float32)[None, :], (128, 128)))
    eye = _bf16(np.eye(128, dtype=np.float32))

    x_np = np.asarray(x, np.float32)
    in_maps = []
    for c in range(NCORES):
        slots = node_of_slot[c * shard:(c + 1) * shard]
        valid = slots >= 0
        xs = np.zeros((shard, cin), np.float32)
        xs[valid] = x_np[slots[valid]]
        x_ct = _bf16(np.ascontiguousarray(xs.T))  # [cin, shard] bf16
        degs = np.ones(shard, np.float32)
        degs[valid] = deg[slots[valid]]
        invd = np.sqrt(degs)
        dis = 1.0 / invd
        dis_pm = np.ascontiguousarray(dis.reshape(blocks, 128).T)
        dis2_pm = np.ascontiguousarray((dis * dis).reshape(blocks, 128).T)
        # [4, (blocks//4)*128]: [k, g*128+p] = invd[(4g+k)*128+p]
        invd_b4 = np.ascontiguousarray(
            invd.reshape(blocks // 4, 4, 128).transpose(1, 0, 2)
            .reshape(4, -1))
        invd_pm = np.ascontiguousarray(invd.reshape(blocks, 128).T)
        in_maps.append({
            "x_ct": x_ct, "dis_pm": dis_pm, "dis2_pm": dis2_pm,
            "invd_b4": invd_b4, "invd_pm": invd_pm,
            "idx_plane": per_core[c]["idx_plane"],
            "rel_plane": _bf16(per_core[c]["rel_plane"]),
            "w1": w1, "w2p": w2p, "b1r": b1r, "b2d": b2d,
            "ones_row": ones_row,
            "iota": iota, "eye": eye,
        })
    return meta, in_maps


def _build_program(cfg, meta):
    import concourse.bacc as bacc
    import concourse.mybir as mybir
    from concourse import tile

    d = meta["d"]
    blocks, shard, qrows, chunk, bpp = (d["blocks"], d["shard"], d["qrows"],
                                        d["chunk"], d["bpp"])
    supers = meta["supers"]
    tiles_bq = meta["tiles_bq"]
    tot_tiles = meta["tot_tiles"]
    batches = meta["batches"]
    cin, chid, cout = cfg["CIN"], cfg["CHID"], cfg["COUT"]

    bf16 = mybir.dt.bfloat16
    f32 = mybir.dt.float32
    i16 = mybir.dt.int16
    mult = mybir.AluOpType.mult
    iseq = mybir.AluOpType.is_equal
    Relu = mybir.ActivationFunctionType.Relu
    Copy = mybir.ActivationFunctionType.Copy

    nc = bacc.Bacc("TRN2", target_bir_lowering=False, debug=False,
                   num_devices=NCORES, num_swdge_queues=4)

    x_ct = nc.dram_tensor("x_ct", [cin, shard], bf16, kind="ExternalInput")
    dis_pm_t = nc.dram_tensor("dis_pm", [128, blocks], f32,
                              kind="ExternalInput")
    dis2_pm_t = nc.dram_tensor("dis2_pm", [128, blocks], f32,
                               kind="ExternalInput")
    invd_b4_t = nc.dram_tensor("invd_b4", [4, (blocks // 4) * 128], f32,
                               kind="ExternalInput")
    invd_pm_t = nc.dram_tensor("invd_pm", [128, blocks], f32,
                               kind="ExternalInput")
    idxp_t = nc.dram_tensor("idx_plane", [128, tot_tiles * 8], i16,
                            kind="ExternalInput")
    relp_t = nc.dram_tensor("rel_plane", [128, tot_tiles], bf16,
                            kind="ExternalInput")
    w1_t = nc.dram_tensor("w1", [cin, chid], bf16, kind="ExternalInput")
    w2p_t = nc.dram_tensor("w2p", [chid, 128], bf16, kind="ExternalInput")
    b1r_t = nc.dram_tensor("b1r", [1, chid], bf16, kind="ExternalInput")
    ones_t = nc.dram_tensor("ones_row", [1, 128], bf16,
                            kind="ExternalInput")
    b2d_t = nc.dram_tensor("b2d", [4, 512], f32, kind="ExternalInput")
    iota_t = nc.dram_tensor("iota", [128, 128], bf16, kind="ExternalInput")
    eye_t = nc.dram_tensor("eye", [128, 128], bf16, kind="ExternalInput")
    out_t = nc.dram_tensor("out", [shard, cout], f32, kind="ExternalOutput")

    # Shared-scratchpad AllGather outputs (faster HBM-HBM collectives)
    tab1 = [nc.dram_tensor(f"t1_{j}", [chunk, chid], bf16,
                           addr_space="Shared") for j in range(PIECES)]
    tab2 = [nc.dram_tensor(f"t2_{j}", [chunk, 128], bf16,
                           addr_space="Shared") for j in range(PIECES)]

    with tile.TileContext(nc) as tc:
        with (
            tc.tile_pool(name="dram", bufs=1, space="DRAM") as dram,
            tc.tile_pool(name="const", bufs=1) as cp,
            tc.tile_pool(name="shards", bufs=1) as shp,
            tc.tile_pool(name="xs", bufs=4) as xp,
            tc.tile_pool(name="stage", bufs=6) as stp,
            tc.tile_pool(name="idxs", bufs=8) as ixp,
            tc.tile_pool(name="masks", bufs=3) as mp,
            tc.tile_pool(name="diag", bufs=4) as dgp,
            tc.tile_pool(name="h1t", bufs=3) as hp,
            tc.tile_pool(name="outp", bufs=4) as op_,
            tc.tile_pool(name="pbig", bufs=4, space="PSUM") as pbig,
            tc.tile_pool(name="pph1", bufs=2, space="PSUM") as pph1,
            tc.tile_pool(name="ppg", bufs=2, space="PSUM") as ppg,
        ):
            # ---- DRAM scratch (collective inputs must be Local) ----
            bounce1 = [dram.tile([qrows, chid], bf16, name=f"bo1_{j}",
                                 tag=f"bo1_{j}") for j in range(PIECES)]
            bounce2 = [dram.tile([qrows, 128], bf16, name=f"bo2_{j}",
                                 tag=f"bo2_{j}") for j in range(PIECES)]

            # ---- constants ----
            iota_sb = cp.tile([128, 128], bf16)
            nc.sync.dma_start(iota_sb[:], iota_t[:])
            eye_sb = cp.tile([128, 128], bf16)
            nc.sync.dma_start(eye_sb[:], eye_t[:])
            w1_sb = cp.tile([cin, chid], bf16)
            nc.sync.dma_start(w1_sb[:], w1_t[:])
            w2_sb = cp.tile([chid, 128], bf16)
            nc.sync.dma_start(w2_sb[:], w2p_t[:])
            b1_sb = cp.tile([1, chid], bf16)
            nc.sync.dma_start(b1_sb[:], b1r_t[:])
            b2_sb = cp.tile([4, 512], f32)
            nc.sync.dma_start(b2_sb[:], b2d_t[:])
            relp_sb = cp.tile([128, tot_tiles], bf16)
            nc.sync.dma_start(relp_sb[:], relp_t[:])

            dis_pm = cp.tile([128, blocks], f32)
            nc.sync.dma_start(dis_pm[:], dis_pm_t[:])
            dis2_pm = cp.tile([128, blocks], f32)
            nc.sync.dma_start(dis2_pm[:], dis2_pm_t[:])
            invd_b4 = cp.tile([4, (blocks // 4) * 128], f32)
            nc.sync.dma_start(invd_b4[:], invd_b4_t[:])
            invd_pm = cp.tile([128, blocks], f32)
            nc.sync.dma_start(invd_pm[:], invd_pm_t[:])
            ones_sb = cp.tile([1, 128], bf16)
            nc.sync.dma_start(ones_sb[:], ones_t[:])
            # b1 broadcast across partitions: [128, chid], row k = b1
            pb1 = pph1.tile([128, chid], f32, tag="ph1")
            nc.tensor.matmul(pb1[:], ones_sb[:], b1_sb[:],
                             start=True, stop=True)
            b1bc = cp.tile([128, chid], bf16)
            nc.scalar.activation(b1bc[:], pb1[:], Copy)

            g1s = shp.tile([128, blocks * chid], bf16)
            g2s = shp.tile([128, blocks * 128], bf16)

            def exchange(bounce, tabs, j):
                nc.gpsimd.collective_compute(
                    "AllGather", mybir.AluOpType.bypass,
                    replica_groups=[list(range(NCORES))],
                    ins=[bounce[j].opt()], outs=[tabs[j][:].opt()])

            # ---- phase 1: dense transform -> g1 shard, exchange per piece
            for j in range(PIECES):
                for b in range(j * bpp, (j + 1) * bpp):
                    xb = xp.tile([cin, 128], bf16, tag="xb")
                    nc.sync.dma_start(xb[:], x_ct[:, b * 128:(b + 1) * 128])
                    pt = pph1.tile([128, chid], f32, tag="ph1")
                    nc.tensor.matmul(pt[:], xb[:], w1_sb[:],
                                     start=True, stop=True)
                    nc.scalar.activation(
                        g1s[:, b * chid:(b + 1) * chid], pt[:], Copy,
                        bias=0.0, scale=dis_pm[:, b:b + 1])
                g1s3 = g1s[:].rearrange("p (b c) -> p b c", c=chid)
                nc.sync.dma_start(
                    bounce1[j][:].rearrange("(b p) c -> p b c", p=128),
                    g1s3[:, j * bpp:(j + 1) * bpp, :])
                exchange(bounce1, tab1, j)

            # ---- gather/aggregate layers ----
            # layer 1: psum [ch, dst]  (lhsT=st, rhs=mask)
            # layer 2: psum [dst, ch]  (lhsT=mask, rhs=st)
            l2x_done = [False] * PIECES

            def l2_exchange_ready(b_done):
                """Fire layer-2 exchanges whose block range is evicted."""
                for j in range(PIECES):
                    if not l2x_done[j] and b_done >= (j + 1) * bpp:
                        g2s3 = g2s[:].rearrange("p (b c) -> p b c", c=128)
                        nc.sync.dma_start(
                            bounce2[j][:].rearrange("(b p) c -> p b c",
                                                    p=128),
                            g2s3[:, j * bpp:(j + 1) * bpp, :])
                        exchange(bounce2, tab2, j)
                        l2x_done[j] = True

            def aggregate(layer, tabs):
                tile_cursor = 0
                batch_i = 0
                b0 = 0
                for g in supers:
                    assert g % 4 == 0
                    nbank = g // 4
                    psums = [pbig.tile([128, 512], f32, name="acc",
                                       tag="acc") for _ in range(nbank)]

                    def pacc(bi):
                        return psums[bi // 4][:, (bi % 4) * 128:
                                              (bi % 4) * 128 + 128]

                    # program-order matmul sequence; find last item per bank
                    seq = [("self", bi) for bi in range(g)]
                    for qq in range(PIECES):
                        for bi in range(g):
                            nt = int(tiles_bq[(b0 + bi) * PIECES + qq])
                            for t in range(nt):
                                seq.append(("edge", qq, bi, t))
                    last_per_bank = {}
                    for item in seq:
                        bi = item[1] if item[0] == "self" else item[2]
                        last_per_bank[bi // 4] = item

                    # seeds
                    for k in range(nbank):
                        gb = (b0 + k * 4) // 4  # global bank index
                        if layer == 1:
                            # psum[ch, dst region] = b1[ch] * invd[dst]
                            for kk in range(4):
                                b = b0 + k * 4 + kk
                                dg = dgp.tile([128, 128], bf16, tag="dg")
                                nc.vector.tensor_scalar(
                                    dg[:], eye_sb[:],
                                    invd_pm[:, b:b + 1], None, mult)
                                nc.tensor.matmul(
                                    psums[k][:, kk * 128:(kk + 1) * 128],
                                    b1bc[:], dg[:],
                                    start=True, stop=False)
                        else:
                            # psum[dst, ch4] = invd[dst] * b2 blockdiag
                            nc.tensor.matmul(
                                psums[k][:],
                                invd_b4[:, gb * 128:(gb + 1) * 128],
                                b2_sb[:], start=True, stop=False)
                    # self loops
                    for bi in range(g):
                        b = b0 + bi
                        stop = last_per_bank[bi // 4] == ("self", bi)
                        if layer == 1:
                            nc.tensor.matmul(
                                pacc(bi), g1s[:, b * chid:(b + 1) * chid],
                                eye_sb[:], start=False, stop=stop)
                        else:
                            nc.tensor.matmul(
                                pacc(bi), eye_sb[:],
                                g2s[:, b * 128:(b + 1) * 128],
                                start=False, stop=stop)
                    # edge tiles, batched per source piece
                    for qq in range(PIECES):
                        nb = batches[batch_i]
                        batch_i += 1
                        if nb == 0:
                            continue
                        idxb = ixp.tile([128, nb * 8], i16, tag="idxb")
                        nc.sync.dma_start(
                            idxb[:], idxp_t[:, tile_cursor * 8:
                                            (tile_cursor + nb) * 8])
                        st = stp.tile([128, nb, 128], bf16, tag="stage")
                        nc.gpsimd.dma_gather(
                            st[:], tabs[qq][:], idxb[:],
                            nb * 128, nb * 128, 128,
                            single_packet=False, queue_num=qq % 4)
                        # one wide mask build for the whole batch
                        mk = mp.tile([128, nb, 128], bf16, tag="mask")
                        iota_b = iota_sb[:].rearrange(
                            "p (t c) -> p t c", t=1).broadcast_to(
                                [128, nb, 128])
                        rel_b = relp_sb[:, tile_cursor:
                                        tile_cursor + nb].rearrange(
                            "p (t o) -> p t o", o=1).broadcast_to(
                                [128, nb, 128])
                        nc.vector.tensor_tensor(mk[:], iota_b, rel_b, iseq)
                        t_local = 0
                        for bi in range(g):
                            b = b0 + bi
                            nt = int(tiles_bq[b * PIECES + qq])
                            for t in range(nt):
                                stop = (last_per_bank[bi // 4] ==
                                        ("edge", qq, bi, t))
                                st_t = st[:, t_local, :].squeeze()
                                mk_t = mk[:, t_local, :].squeeze()
                                if layer == 1:
                                    nc.tensor.matmul(pacc(bi), st_t, mk_t,
                                                     start=False, stop=stop)
                                else:
                                    nc.tensor.matmul(pacc(bi), mk_t, st_t,
                                                     start=False, stop=stop)
                                t_local += 1
                        tile_cursor += nb
                    # evictions
                    if layer == 1:
                        for k in range(nbank):
                            h1b = hp.tile([128, 512], bf16, tag="h1b")
                            nc.scalar.activation(h1b[:], psums[k][:], Relu)
                            for kk in range(4):
                                bi = k * 4 + kk
                                b = b0 + bi
                                pg = ppg.tile([128, 128], f32, tag="pg")
                                nc.tensor.matmul(
                                    pg[:], h1b[:, kk * 128:(kk + 1) * 128],
                                    w2_sb[:], start=True, stop=True)
                                nc.scalar.activation(
                                    g2s[:, b * 128:(b + 1) * 128], pg[:],
                                    Copy, bias=0.0,
                                    scale=dis2_pm[:, b:b + 1])
                        l2_exchange_ready(b0 + g)
                    else:
                        for bi in range(g):
                            b = b0 + bi
                            ob = op_.tile([128, cout], f32, tag="ob")
                            nc.scalar.activation(
                                ob[:], pacc(bi)[:, :cout], Copy, bias=0.0,
                                scale=dis_pm[:, b:b + 1])
                            nc.sync.dma_start(
                                out_t[b * 128:(b + 1) * 128, :], ob[:])
                    b0 += g

            aggregate(1, tab1)
            aggregate(2, tab2)

    nc.compile()
    return nc


def run_config(inputs, cfg, run=None):
    from concourse.bass_utils import run_bass_kernel_spmd

    x = np.asarray(inputs["x"], np.float32)
    edge_index = np.asarray(inputs["edge_index"])
    meta, in_maps = _host_inputs(
        x, edge_index, inputs["W1"], inputs["b1"], inputs["W2"],
        inputs["b2"], cfg)
    nc = _build_program(cfg, meta)
    if run is None:
        def run(nc, in_maps):
            return run_bass_kernel_spmd(
                nc, in_maps, list(range(NCORES))).results
    results = run(nc, in_maps)
    d = _derive(cfg)
    shard = d["shard"]
    all_rows = np.concatenate(
        [results[c]["out"] for c in range(NCORES)], axis=0)
    node_of_slot = meta["node_of_slot"]
    valid = node_of_slot >= 0
    out = np.empty((cfg["N"], cfg["COUT"]), np.float32)
    out[node_of_slot[valid]] = all_rows[valid]
    return np.ascontiguousarray(out)


def kernel(**inputs):
    return run_config(inputs, CFG_FULL)


# revision 6
# speedup vs baseline: 3.2423x; 1.2343x over previous
"""Two-layer GCN (GCNConv x2, PyG-style symmetric normalization) on 8 trn2
NeuronCores.

Vertex-cut graph parallelism, v2:
  - Nodes are assigned to (core, block, slot) positions by a host-side
    LPT balancer so per-(block, piece) edge counts are nearly uniform
    across cores (the SPMD tile schedule is shared by all cores, so the
    max over cores determines the padded tile count).
  - Normalization is factored:  out[d] = b + dis[d]*(sum_{e:col=d}
    g[row_e] + g[d]),  g[n] = dis[n]*(x@W)[n],  dis = 1/sqrt(deg).
    Aggregation is a pure indicator matmul over 128-edge tiles.
  - Layer-1 accumulates transposed ([ch, dst] in PSUM) so the relu
    eviction needs no PE transpose: relu is done unscaled on the Scalar
    engine (relu(dis*x) = dis*relu(x), dis>0) and the dis^2 factor is
    folded into the post-W2 scale.
  - One-hot masks are built in one wide DVE tensor_tensor per gather
    batch using stride-0 broadcast APs (iota == rel), instead of one
    tensor_scalar per tile.
  - Gathers run on 4 SWDGE queues (one per table piece) so batches
    drain concurrently on the DMA engines.
  - All PSUM evictions run on the Scalar (ACT) engine; DVE only builds
    masks; GpSimd only generates gather descriptors and triggers
    collectives.
  - Layer-2 table pieces are exchanged as soon as their block range has
    been evicted, overlapping the AllGathers with layer-1 aggregation.
"""

import math

import numpy as np

try:
    from ml_dtypes import bfloat16 as np_bf16
except ImportError:  # pragma: no cover
    np_bf16 = None

CFG_FULL = dict(N=100000, E=1600000, CIN=128, CHID=128, COUT=64)

NCORES = 8
PIECES = 4  # table pieces / AllGather splits (int16 gather index limit)
SUPER = 4  # dst blocks per gather batch group


def _derive(cfg):
    n = cfg["N"]
    bucket = n // NCORES
    assert bucket * NCORES == n
    blocks = math.ceil(bucket / 128)
    blocks = math.ceil(blocks / (4 * PIECES)) * (4 * PIECES)
    shard = blocks * 128
    bpp = blocks // PIECES  # blocks per piece
    qrows = shard // PIECES  # rows per piece per core
    chunk = qrows * NCORES  # rows of one assembled table piece
    assert chunk <= 32600, chunk  # int16 gather index limit
    supers = [SUPER] * (blocks // SUPER)
    if blocks % SUPER:
        supers.append(blocks % SUPER)
    return dict(bucket=bucket, blocks=blocks, shard=shard, qrows=qrows,
                chunk=chunk, supers=supers, bpp=bpp)


def _assign_nodes(edge_index, cfg):
    """LPT-balance destination load: node -> (core, block, slot-in-block).

    Returns slot_of_node [N] (global slot id in 0..NCORES*shard) and
    node_of_slot [NCORES*shard] (-1 for padding slots).
    """
    d = _derive(cfg)
    n, blocks, shard = cfg["N"], d["blocks"], d["shard"]
    nbins = NCORES * blocks
    deg_in = np.bincount(edge_index[1], minlength=n).astype(np.int64)
    order = np.argsort(-deg_in, kind="stable")
    cap = math.ceil(n / nbins)
    assert cap <= 128
    load = np.zeros(nbins, np.int64)
    slot_of_node = np.empty(n, np.int64)
    fill = np.zeros(nbins, np.int64)
    for r in range(cap):
        chunk_nodes = order[r * nbins:(r + 1) * nbins]
        if chunk_nodes.size == 0:
            break
        bins = np.argsort(load, kind="stable")[:chunk_nodes.size]
        load[bins] += deg_in[chunk_nodes]
        core = bins // blocks
        blk = bins % blocks
        slot_of_node[chunk_nodes] = core * shard + blk * 128 + fill[bins]
        fill[bins] += 1
    node_of_slot = np.full(NCORES * shard, -1, np.int64)
    node_of_slot[slot_of_node] = np.arange(n)
    return slot_of_node, node_of_slot


def _preprocess(edge_index, slot_of_node, cfg):
    """Bucket & sort edges (by destination core/block/piece), build
    per-core gather/mask planes with the shared tile schedule."""
    d = _derive(cfg)
    blocks, qrows, shard = d["blocks"], d["qrows"], d["shard"]
    src = slot_of_node[edge_index[0].astype(np.int64)]
    dst = slot_of_node[edge_index[1].astype(np.int64)]

    c_dst = dst // shard
    d_l = dst - c_dst * shard
    blk = d_l // 128
    rel = (d_l % 128).astype(np.float32)
    c_src = src // shard
    r_l = src - c_src * shard
    q = r_l // qrows
    ric = (c_src * qrows + r_l % qrows).astype(np.int64)  # row in chunk q

    nbq = blocks * PIECES
    key_bq = blk * PIECES + q
    counts = np.zeros((NCORES, nbq), np.int64)
    for c in range(NCORES):
        m = c_dst == c
        counts[c] = np.bincount(key_bq[m], minlength=nbq)
    tiles_bq = np.ceil(counts.max(axis=0) / 128).astype(np.int64)  # [nbq]

    # tile schedule in program order: (super, piece, block in super, tile)
    order_bq = []
    supers = d["supers"]
    b0 = 0
    batches = []  # tiles per (super, piece) gather batch
    for g in supers:
        for qq in range(PIECES):
            nt = 0
            for b in range(b0, b0 + g):
                order_bq.append((b, qq))
                nt += int(tiles_bq[b * PIECES + qq])
            batches.append(nt)
        b0 += g
    tot_tiles = int(tiles_bq.sum())
    assert sum(batches) == tot_tiles and tot_tiles > 0

    off_bq = np.zeros(nbq, np.int64)
    acc = 0
    for (b, qq) in order_bq:
        off_bq[b * PIECES + qq] = acc
        acc += int(tiles_bq[b * PIECES + qq])

    per_core = []
    for c in range(NCORES):
        m = c_dst == c
        okey = (blk[m] * PIECES + q[m]).astype(np.int64)
        sort = np.argsort(okey, kind="stable")
        okey_s = okey[sort]
        e_rel = rel[m][sort]
        e_ric = ric[m][sort]
        slot_base = off_bq[okey_s] * 128
        grp_start = np.searchsorted(okey_s, okey_s)
        within = np.arange(okey_s.size) - grp_start
        slots = slot_base + within
        idx_flat = np.zeros(tot_tiles * 128, np.int16)
        rel_flat = np.full(tot_tiles * 128, -1.0, np.float32)
        idx_flat[slots] = e_ric.astype(np.int16)
        rel_flat[slots] = e_rel
        idx16 = idx_flat.reshape(tot_tiles * 8, 16).T  # [16, tiles*8]
        idx_plane = np.tile(idx16, (8, 1)).copy()
        rel_plane = np.ascontiguousarray(
            rel_flat.reshape(tot_tiles, 128).T)  # [128, tot_tiles]
        per_core.append(dict(idx_plane=idx_plane, rel_plane=rel_plane))

    meta = dict(d=d, tiles_bq=tiles_bq, batches=batches, tot_tiles=tot_tiles,
                supers=supers)
    return meta, per_core


def _bf16(a):
    a = np.asarray(a, np.float32)
    if np_bf16 is not None:
        return a.astype(np_bf16)
    return a  # fall back: ship f32 (kernel would need dtype change)


def _host_inputs(x, edge_index, W1, b1, W2, b2, cfg):
    d = _derive(cfg)
    blocks, shard = d["blocks"], d["shard"]
    n, cin = cfg["N"], cfg["CIN"]
    chid, cout = cfg["CHID"], cfg["COUT"]
    slot_of_node, node_of_slot = _assign_nodes(edge_index, cfg)
    meta, per_core = _preprocess(edge_index, slot_of_node, cfg)
    meta["node_of_slot"] = node_of_slot

    col = edge_index[1].astype(np.int64)
    deg = (np.bincount(col, minlength=n) + 1).astype(np.float32)

    w1 = _bf16(W1)
    w2p = np.zeros((chid, 128), np.float32)
    w2p[:, :cout] = np.asarray(W2, np.float32)
    w2p = _bf16(w2p)
    b1r = _bf16(np.asarray(b1, np.float32).reshape(1, chid))
    ones_row = _bf16(np.ones((1, 128), np.float32))
    b2d = np.zeros((4, 512), np.float32)
    for k in range(4):
        b2d[k, k * 128:k * 128 + cout] = np.asarray(b2, np.float32)
    iota = _bf16(np.broadcast_to(
        np.arange(128, dtype=np.float32)[None, :], (128, 128)))
    eye = _bf16(np.eye(128, dtype=np.float32))

    x_np = np.asarray(x, np.float32)
    in_maps = []
    for c in range(NCORES):
        slots = node_of_slot[c * shard:(c + 1) * shard]
        valid = slots >= 0
        xs = np.zeros((shard, cin), np.float32)
        xs[valid] = x_np[slots[valid]]
        x_ct = _bf16(np.ascontiguousarray(xs.T))  # [cin, shard] bf16
        degs = np.ones(shard, np.float32)
        degs[valid] = deg[slots[valid]]
        invd = np.sqrt(degs)
        dis = 1.0 / invd
        dis_pm = np.ascontiguousarray(dis.reshape(blocks, 128).T)
        dis2_pm = np.ascontiguousarray((dis * dis).reshape(blocks, 128).T)
        # [4, (blocks//4)*128]: [k, g*128+p] = invd[(4g+k)*128+p]
        invd_b4 = np.ascontiguousarray(
            invd.reshape(blocks // 4, 4, 128).transpose(1, 0, 2)
            .reshape(4, -1))
        invd_pm = np.ascontiguousarray(invd.reshape(blocks, 128).T)
        in_maps.append({
            "x_ct": x_ct, "dis_pm": dis_pm, "dis2_pm": dis2_pm,
            "invd_b4": invd_b4, "invd_pm": invd_pm,
            "idx_plane": per_core[c]["idx_plane"],
            "rel_plane": _bf16(per_core[c]["rel_plane"]),
            "w1": w1, "w2p": w2p, "b1r": b1r, "b2d": b2d,
            "ones_row": ones_row,
            "iota": iota, "eye": eye,
        })
    return meta, in_maps


def _build_program(cfg, meta):
    import concourse.bacc as bacc
    import concourse.mybir as mybir
    from concourse import tile

    d = meta["d"]
    blocks, shard, qrows, chunk, bpp = (d["blocks"], d["shard"], d["qrows"],
                                        d["chunk"], d["bpp"])
    supers = meta["supers"]
    tiles_bq = meta["tiles_bq"]
    tot_tiles = meta["tot_tiles"]
    batches = meta["batches"]
    cin, chid, cout = cfg["CIN"], cfg["CHID"], cfg["COUT"]

    bf16 = mybir.dt.bfloat16
    f32 = mybir.dt.float32
    i16 = mybir.dt.int16
    mult = mybir.AluOpType.mult
    iseq = mybir.AluOpType.is_equal
    Relu = mybir.ActivationFunctionType.Relu
    Copy = mybir.ActivationFunctionType.Copy

    nc = bacc.Bacc("TRN2", target_bir_lowering=False, debug=False,
                   num_devices=NCORES, num_swdge_queues=4)

    x_ct = nc.dram_tensor("x_ct", [cin, shard], bf16, kind="ExternalInput")
    dis_pm_t = nc.dram_tensor("dis_pm", [128, blocks], f32,
                              kind="ExternalInput")
    dis2_pm_t = nc.dram_tensor("dis2_pm", [128, blocks], f32,
                               kind="ExternalInput")
    invd_b4_t = nc.dram_tensor("invd_b4", [4, (blocks // 4) * 128], f32,
                               kind="ExternalInput")
    invd_pm_t = nc.dram_tensor("invd_pm", [128, blocks], f32,
                               kind="ExternalInput")
    idxp_t = nc.dram_tensor("idx_plane", [128, tot_tiles * 8], i16,
                            kind="ExternalInput")
    relp_t = nc.dram_tensor("rel_plane", [128, tot_tiles], bf16,
                            kind="ExternalInput")
    w1_t = nc.dram_tensor("w1", [cin, chid], bf16, kind="ExternalInput")
    w2p_t = nc.dram_tensor("w2p", [chid, 128], bf16, kind="ExternalInput")
    b1r_t = nc.dram_tensor("b1r", [1, chid], bf16, kind="ExternalInput")
    ones_t = nc.dram_tensor("ones_row", [1, 128], bf16,
                            kind="ExternalInput")
    b2d_t = nc.dram_tensor("b2d", [4, 512], f32, kind="ExternalInput")
    iota_t = nc.dram_tensor("iota", [128, 128], bf16, kind="ExternalInput")
    eye_t = nc.dram_tensor("eye", [128, 128], bf16, kind="ExternalInput")
    out_t = nc.dram_tensor("out", [shard, cout], f32, kind="ExternalOutput")

    # Shared-scratchpad AllGather outputs (faster HBM-HBM collectives)
    tab1 = [nc.dram_tensor(f"t1_{j}", [chunk, chid], bf16,
                           addr_space="Shared") for j in range(PIECES)]
    tab2 = [nc.dram_tensor(f"t2_{j}", [chunk, 128], bf16,
                           addr_space="Shared") for j in range(PIECES)]

    with tile.TileContext(nc) as tc:
        with (
            tc.tile_pool(name="dram", bufs=1, space="DRAM") as dram,
            tc.tile_pool(name="const", bufs=1) as cp,
            tc.tile_pool(name="shards", bufs=1) as shp,
            tc.tile_pool(name="xs", bufs=4) as xp,
            tc.tile_pool(name="stage", bufs=10) as stp,
            tc.tile_pool(name="idxs", bufs=12) as ixp,
            tc.tile_pool(name="masks", bufs=5) as mp,
            tc.tile_pool(name="diag", bufs=4) as dgp,
            tc.tile_pool(name="h1t", bufs=3) as hp,
            tc.tile_pool(name="outp", bufs=4) as op_,
            tc.tile_pool(name="pbig", bufs=4, space="PSUM") as pbig,
            tc.tile_pool(name="pph1", bufs=2, space="PSUM") as pph1,
            tc.tile_pool(name="ppg", bufs=2, space="PSUM") as ppg,
        ):
            # ---- DRAM scratch (collective inputs must be Local) ----
            bounce1 = [dram.tile([qrows, chid], bf16, name=f"bo1_{j}",
                                 tag=f"bo1_{j}") for j in range(PIECES)]
            bounce2 = [dram.tile([qrows, 128], bf16, name=f"bo2_{j}",
                                 tag=f"bo2_{j}") for j in range(PIECES)]

            # ---- constants ----
            iota_sb = cp.tile([128, 128], bf16)
            nc.sync.dma_start(iota_sb[:], iota_t[:])
            eye_sb = cp.tile([128, 128], bf16)
            nc.sync.dma_start(eye_sb[:], eye_t[:])
            w1_sb = cp.tile([cin, chid], bf16)
            nc.sync.dma_start(w1_sb[:], w1_t[:])
            w2_sb = cp.tile([chid, 128], bf16)
            nc.sync.dma_start(w2_sb[:], w2p_t[:])
            b1_sb = cp.tile([1, chid], bf16)
            nc.sync.dma_start(b1_sb[:], b1r_t[:])
            b2_sb = cp.tile([4, 512], f32)
            nc.sync.dma_start(b2_sb[:], b2d_t[:])
            relp_sb = cp.tile([128, tot_tiles], bf16)
            nc.sync.dma_start(relp_sb[:], relp_t[:])

            dis_pm = cp.tile([128, blocks], f32)
            nc.sync.dma_start(dis_pm[:], dis_pm_t[:])
            dis2_pm = cp.tile([128, blocks], f32)
            nc.sync.dma_start(dis2_pm[:], dis2_pm_t[:])
            invd_b4 = cp.tile([4, (blocks // 4) * 128], f32)
            nc.sync.dma_start(invd_b4[:], invd_b4_t[:])
            invd_pm = cp.tile([128, blocks], f32)
            nc.sync.dma_start(invd_pm[:], invd_pm_t[:])
            ones_sb = cp.tile([1, 128], bf16)
            nc.sync.dma_start(ones_sb[:], ones_t[:])
            # b1 broadcast across partitions: [128, chid], row k = b1
            pb1 = pph1.tile([128, chid], f32, tag="ph1")
            nc.tensor.matmul(pb1[:], ones_sb[:], b1_sb[:],
                             start=True, stop=True)
            b1bc = cp.tile([128, chid], bf16)
            nc.scalar.activation(b1bc[:], pb1[:], Copy)

            g1s = shp.tile([128, blocks * chid], bf16)
            g2s = shp.tile([128, blocks * 128], bf16)

            def exchange(bounce, tabs, j):
                nc.gpsimd.collective_compute(
                    "AllGather", mybir.AluOpType.bypass,
                    replica_groups=[list(range(NCORES))],
                    ins=[bounce[j].opt()], outs=[tabs[j][:].opt()])

            # ---- phase 1: dense transform -> g1 shard, exchange per piece
            for j in range(PIECES):
                for b in range(j * bpp, (j + 1) * bpp):
                    xb = xp.tile([cin, 128], bf16, tag="xb")
                    nc.sync.dma_start(xb[:], x_ct[:, b * 128:(b + 1) * 128])
                    pt = pph1.tile([128, chid], f32, tag="ph1")
                    nc.tensor.matmul(pt[:], xb[:], w1_sb[:],
                                     start=True, stop=True)
                    nc.scalar.activation(
                        g1s[:, b * chid:(b + 1) * chid], pt[:], Copy,
                        bias=0.0, scale=dis_pm[:, b:b + 1])
                g1s3 = g1s[:].rearrange("p (b c) -> p b c", c=chid)
                nc.sync.dma_start(
                    bounce1[j][:].rearrange("(b p) c -> p b c", p=128),
                    g1s3[:, j * bpp:(j + 1) * bpp, :])
                exchange(bounce1, tab1, j)

            # ---- gather/aggregate layers ----
            # layer 1: psum [ch, dst]  (lhsT=st, rhs=mask)
            # layer 2: psum [dst, ch]  (lhsT=mask, rhs=st)
            l2x_done = [False] * PIECES

            def l2_exchange_ready(b_done):
                """Fire layer-2 exchanges whose block range is evicted."""
                for j in range(PIECES):
                    if not l2x_done[j] and b_done >= (j + 1) * bpp:
                        g2s3 = g2s[:].rearrange("p (b c) -> p b c", c=128)
                        nc.sync.dma_start(
                            bounce2[j][:].rearrange("(b p) c -> p b c",
                                                    p=128),
                            g2s3[:, j * bpp:(j + 1) * bpp, :])
                        exchange(bounce2, tab2, j)
                        l2x_done[j] = True

            def aggregate(layer, tabs):
                tile_cursor = 0
                batch_i = 0
                b0 = 0
                for g in supers:
                    assert g % 4 == 0
                    nbank = g // 4
                    psums = [pbig.tile([128, 512], f32, name="acc",
                                       tag="acc") for _ in range(nbank)]

                    def pacc(bi):
                        return psums[bi // 4][:, (bi % 4) * 128:
                                              (bi % 4) * 128 + 128]

                    # program-order matmul sequence; find last item per bank
                    seq = [("self", bi) for bi in range(g)]
                    for qq in range(PIECES):
                        for bi in range(g):
                            nt = int(tiles_bq[(b0 + bi) * PIECES + qq])
                            for t in range(nt):
                                seq.append(("edge", qq, bi, t))
                    last_per_bank = {}
                    for item in seq:
                        bi = item[1] if item[0] == "self" else item[2]
                        last_per_bank[bi // 4] = item

                    # seeds
                    for k in range(nbank):
                        gb = (b0 + k * 4) // 4  # global bank index
                        if layer == 1:
                            # psum[ch, dst region] = b1[ch] * invd[dst]
                            for kk in range(4):
                                b = b0 + k * 4 + kk
                                dg = dgp.tile([128, 128], bf16, tag="dg")
                                nc.gpsimd.tensor_scalar(
                                    dg[:], eye_sb[:],
                                    invd_pm[:, b:b + 1], None, mult)
                                nc.tensor.matmul(
                                    psums[k][:, kk * 128:(kk + 1) * 128],
                                    b1bc[:], dg[:],
                                    start=True, stop=False)
                        else:
                            # psum[dst, ch4] = invd[dst] * b2 blockdiag
                            nc.tensor.matmul(
                                psums[k][:],
                                invd_b4[:, gb * 128:(gb + 1) * 128],
                                b2_sb[:], start=True, stop=False)
                    # self loops
                    for bi in range(g):
                        b = b0 + bi
                        stop = last_per_bank[bi // 4] == ("self", bi)
                        if layer == 1:
                            nc.tensor.matmul(
                                pacc(bi), g1s[:, b * chid:(b + 1) * chid],
                                eye_sb[:], start=False, stop=stop)
                        else:
                            nc.tensor.matmul(
                                pacc(bi), eye_sb[:],
                                g2s[:, b * 128:(b + 1) * 128],
                                start=False, stop=stop)
                    # edge tiles, batched per source piece
                    for qq in range(PIECES):
                        nb = batches[batch_i]
                        batch_i += 1
                        if nb == 0:
                            continue
                        idxb = ixp.tile([128, nb * 8], i16, tag="idxb")
                        nc.sync.dma_start(
                            idxb[:], idxp_t[:, tile_cursor * 8:
                                            (tile_cursor + nb) * 8])
                        st = stp.tile([128, nb, 128], bf16, tag="stage")
                        nc.gpsimd.dma_gather(
                            st[:], tabs[qq][:], idxb[:],
                            nb * 128, nb * 128, 128,
                            single_packet=False, queue_num=qq % 4)
                        # one wide mask build for the whole batch
                        mk = mp.tile([128, nb, 128], bf16, tag="mask")
                        iota_b = iota_sb[:].rearrange(
                            "p (t c) -> p t c", t=1).broadcast_to(
                                [128, nb, 128])
                        rel_b = relp_sb[:, tile_cursor:
                                        tile_cursor + nb].rearrange(
                            "p (t o) -> p t o", o=1).broadcast_to(
                                [128, nb, 128])
                        nc.vector.tensor_tensor(mk[:], iota_b, rel_b, iseq)
                        t_local = 0
                        for bi in range(g):
                            b = b0 + bi
                            nt = int(tiles_bq[b * PIECES + qq])
                            for t in range(nt):
                                stop = (last_per_bank[bi // 4] ==
                                        ("edge", qq, bi, t))
                                st_t = st[:, t_local, :].squeeze()
                                mk_t = mk[:, t_local, :].squeeze()
                                if layer == 1:
                                    nc.tensor.matmul(pacc(bi), st_t, mk_t,
                                                     start=False, stop=stop)
                                else:
                                    nc.tensor.matmul(pacc(bi), mk_t, st_t,
                                                     start=False, stop=stop)
                                t_local += 1
                        tile_cursor += nb
                    # evictions
                    if layer == 1:
                        for k in range(nbank):
                            h1b = hp.tile([128, 512], bf16, tag="h1b")
                            nc.scalar.activation(h1b[:], psums[k][:], Relu)
                            for kk in range(4):
                                bi = k * 4 + kk
                                b = b0 + bi
                                pg = ppg.tile([128, 128], f32, tag="pg")
                                nc.tensor.matmul(
                                    pg[:], h1b[:, kk * 128:(kk + 1) * 128],
                                    w2_sb[:], start=True, stop=True)
                                nc.scalar.activation(
                                    g2s[:, b * 128:(b + 1) * 128], pg[:],
                                    Copy, bias=0.0,
                                    scale=dis2_pm[:, b:b + 1])
                        l2_exchange_ready(b0 + g)
                    else:
                        for bi in range(g):
                            b = b0 + bi
                            ob = op_.tile([128, cout], f32, tag="ob")
                            nc.scalar.activation(
                                ob[:], pacc(bi)[:, :cout], Copy, bias=0.0,
                                scale=dis_pm[:, b:b + 1])
                            nc.sync.dma_start(
                                out_t[b * 128:(b + 1) * 128, :], ob[:])
                    b0 += g

            aggregate(1, tab1)
            aggregate(2, tab2)

    nc.compile()
    return nc


def run_config(inputs, cfg, run=None):
    from concourse.bass_utils import run_bass_kernel_spmd

    x = np.asarray(inputs["x"], np.float32)
    edge_index = np.asarray(inputs["edge_index"])
    meta, in_maps = _host_inputs(
        x, edge_index, inputs["W1"], inputs["b1"], inputs["W2"],
        inputs["b2"], cfg)
    nc = _build_program(cfg, meta)
    if run is None:
        def run(nc, in_maps):
            return run_bass_kernel_spmd(
                nc, in_maps, list(range(NCORES))).results
    results = run(nc, in_maps)
    d = _derive(cfg)
    shard = d["shard"]
    all_rows = np.concatenate(
        [results[c]["out"] for c in range(NCORES)], axis=0)
    node_of_slot = meta["node_of_slot"]
    valid = node_of_slot >= 0
    out = np.empty((cfg["N"], cfg["COUT"]), np.float32)
    out[node_of_slot[valid]] = all_rows[valid]
    return np.ascontiguousarray(out)


def kernel(**inputs):
    return run_config(inputs, CFG_FULL)


# revision 7
# speedup vs baseline: 3.2474x; 1.0016x over previous
"""Two-layer GCN (GCNConv x2, PyG-style symmetric normalization) on 8 trn2
NeuronCores.

Vertex-cut graph parallelism, v2:
  - Nodes are assigned to (core, block, slot) positions by a host-side
    LPT balancer so per-(block, piece) edge counts are nearly uniform
    across cores (the SPMD tile schedule is shared by all cores, so the
    max over cores determines the padded tile count).
  - Normalization is factored:  out[d] = b + dis[d]*(sum_{e:col=d}
    g[row_e] + g[d]),  g[n] = dis[n]*(x@W)[n],  dis = 1/sqrt(deg).
    Aggregation is a pure indicator matmul over 128-edge tiles.
  - Layer-1 accumulates transposed ([ch, dst] in PSUM) so the relu
    eviction needs no PE transpose: relu is done unscaled on the Scalar
    engine (relu(dis*x) = dis*relu(x), dis>0) and the dis^2 factor is
    folded into the post-W2 scale.
  - One-hot masks are built in one wide DVE tensor_tensor per gather
    batch using stride-0 broadcast APs (iota == rel), instead of one
    tensor_scalar per tile.
  - Gathers run on 4 SWDGE queues (one per table piece) so batches
    drain concurrently on the DMA engines.
  - All PSUM evictions run on the Scalar (ACT) engine; DVE only builds
    masks; GpSimd only generates gather descriptors and triggers
    collectives.
  - Layer-2 table pieces are exchanged as soon as their block range has
    been evicted, overlapping the AllGathers with layer-1 aggregation.
"""

import math

import numpy as np

try:
    from ml_dtypes import bfloat16 as np_bf16
except ImportError:  # pragma: no cover
    np_bf16 = None

CFG_FULL = dict(N=100000, E=1600000, CIN=128, CHID=128, COUT=64)

NCORES = 8
PIECES = 4  # table pieces / AllGather splits (int16 gather index limit)
SUPER = 4  # dst blocks per gather batch group


def _derive(cfg):
    n = cfg["N"]
    bucket = n // NCORES
    assert bucket * NCORES == n
    blocks = math.ceil(bucket / 128)
    blocks = math.ceil(blocks / (4 * PIECES)) * (4 * PIECES)
    shard = blocks * 128
    bpp = blocks // PIECES  # blocks per piece
    qrows = shard // PIECES  # rows per piece per core
    chunk = qrows * NCORES  # rows of one assembled table piece
    assert chunk <= 32600, chunk  # int16 gather index limit
    supers = [SUPER] * (blocks // SUPER)
    if blocks % SUPER:
        supers.append(blocks % SUPER)
    return dict(bucket=bucket, blocks=blocks, shard=shard, qrows=qrows,
                chunk=chunk, supers=supers, bpp=bpp)


def _assign_nodes(edge_index, cfg):
    """LPT-balance destination load: node -> (core, block, slot-in-block).

    Returns slot_of_node [N] (global slot id in 0..NCORES*shard) and
    node_of_slot [NCORES*shard] (-1 for padding slots).
    """
    d = _derive(cfg)
    n, blocks, shard = cfg["N"], d["blocks"], d["shard"]
    nbins = NCORES * blocks
    deg_in = np.bincount(edge_index[1], minlength=n).astype(np.int64)
    order = np.argsort(-deg_in, kind="stable")
    cap = math.ceil(n / nbins)
    assert cap <= 128
    load = np.zeros(nbins, np.int64)
    slot_of_node = np.empty(n, np.int64)
    fill = np.zeros(nbins, np.int64)
    for r in range(cap):
        chunk_nodes = order[r * nbins:(r + 1) * nbins]
        if chunk_nodes.size == 0:
            break
        bins = np.argsort(load, kind="stable")[:chunk_nodes.size]
        load[bins] += deg_in[chunk_nodes]
        core = bins // blocks
        blk = bins % blocks
        slot_of_node[chunk_nodes] = core * shard + blk * 128 + fill[bins]
        fill[bins] += 1
    node_of_slot = np.full(NCORES * shard, -1, np.int64)
    node_of_slot[slot_of_node] = np.arange(n)
    return slot_of_node, node_of_slot


def _preprocess(edge_index, slot_of_node, cfg):
    """Bucket & sort edges (by destination core/block/piece), build
    per-core gather/mask planes with the shared tile schedule."""
    d = _derive(cfg)
    blocks, qrows, shard = d["blocks"], d["qrows"], d["shard"]
    src = slot_of_node[edge_index[0].astype(np.int64)]
    dst = slot_of_node[edge_index[1].astype(np.int64)]

    c_dst = dst // shard
    d_l = dst - c_dst * shard
    blk = d_l // 128
    rel = (d_l % 128).astype(np.float32)
    c_src = src // shard
    r_l = src - c_src * shard
    q = r_l // qrows
    ric = (c_src * qrows + r_l % qrows).astype(np.int64)  # row in chunk q

    nbq = blocks * PIECES
    key_bq = blk * PIECES + q
    counts = np.zeros((NCORES, nbq), np.int64)
    for c in range(NCORES):
        m = c_dst == c
        counts[c] = np.bincount(key_bq[m], minlength=nbq)
    tiles_bq = np.ceil(counts.max(axis=0) / 128).astype(np.int64)  # [nbq]

    # tile schedule in program order: (super, piece, block in super, tile)
    order_bq = []
    supers = d["supers"]
    b0 = 0
    batches = []  # tiles per (super, piece) gather batch
    for g in supers:
        for qq in range(PIECES):
            nt = 0
            for b in range(b0, b0 + g):
                order_bq.append((b, qq))
                nt += int(tiles_bq[b * PIECES + qq])
            batches.append(nt)
        b0 += g
    tot_tiles = int(tiles_bq.sum())
    assert sum(batches) == tot_tiles and tot_tiles > 0

    off_bq = np.zeros(nbq, np.int64)
    acc = 0
    for (b, qq) in order_bq:
        off_bq[b * PIECES + qq] = acc
        acc += int(tiles_bq[b * PIECES + qq])

    per_core = []
    for c in range(NCORES):
        m = c_dst == c
        okey = (blk[m] * PIECES + q[m]).astype(np.int64)
        sort = np.argsort(okey, kind="stable")
        okey_s = okey[sort]
        e_rel = rel[m][sort]
        e_ric = ric[m][sort]
        slot_base = off_bq[okey_s] * 128
        grp_start = np.searchsorted(okey_s, okey_s)
        within = np.arange(okey_s.size) - grp_start
        slots = slot_base + within
        idx_flat = np.zeros(tot_tiles * 128, np.int16)
        rel_flat = np.full(tot_tiles * 128, -1.0, np.float32)
        idx_flat[slots] = e_ric.astype(np.int16)
        rel_flat[slots] = e_rel
        idx16 = idx_flat.reshape(tot_tiles * 8, 16).T  # [16, tiles*8]
        idx_plane = np.tile(idx16, (8, 1)).copy()
        rel_plane = np.ascontiguousarray(
            rel_flat.reshape(tot_tiles, 128).T)  # [128, tot_tiles]
        per_core.append(dict(idx_plane=idx_plane, rel_plane=rel_plane))

    meta = dict(d=d, tiles_bq=tiles_bq, batches=batches, tot_tiles=tot_tiles,
                supers=supers)
    return meta, per_core


def _bf16(a):
    a = np.asarray(a, np.float32)
    if np_bf16 is not None:
        return a.astype(np_bf16)
    return a  # fall back: ship f32 (kernel would need dtype change)


def _host_inputs(x, edge_index, W1, b1, W2, b2, cfg):
    d = _derive(cfg)
    blocks, shard = d["blocks"], d["shard"]
    n, cin = cfg["N"], cfg["CIN"]
    chid, cout = cfg["CHID"], cfg["COUT"]
    slot_of_node, node_of_slot = _assign_nodes(edge_index, cfg)
    meta, per_core = _preprocess(edge_index, slot_of_node, cfg)
    meta["node_of_slot"] = node_of_slot

    col = edge_index[1].astype(np.int64)
    deg = (np.bincount(col, minlength=n) + 1).astype(np.float32)

    w1 = _bf16(W1)
    w2p = np.zeros((chid, 128), np.float32)
    w2p[:, :cout] = np.asarray(W2, np.float32)
    w2p = _bf16(w2p)
    b1r = _bf16(np.asarray(b1, np.float32).reshape(1, chid))
    ones_row = _bf16(np.ones((1, 128), np.float32))
    b2d = np.zeros((4, 512), np.float32)
    for k in range(4):
        b2d[k, k * 128:k * 128 + cout] = np.asarray(b2, np.float32)
    iota = _bf16(np.broadcast_to(
        np.arange(128, dtype=np.float32)[None, :], (128, 128)))
    eye = _bf16(np.eye(128, dtype=np.float32))

    x_np = np.asarray(x, np.float32)
    in_maps = []
    for c in range(NCORES):
        slots = node_of_slot[c * shard:(c + 1) * shard]
        valid = slots >= 0
        xs = np.zeros((shard, cin), np.float32)
        xs[valid] = x_np[slots[valid]]
        x_ct = _bf16(np.ascontiguousarray(xs.T))  # [cin, shard] bf16
        degs = np.ones(shard, np.float32)
        degs[valid] = deg[slots[valid]]
        invd = np.sqrt(degs)
        dis = 1.0 / invd
        dis_pm = np.ascontiguousarray(dis.reshape(blocks, 128).T)
        dis2_pm = np.ascontiguousarray((dis * dis).reshape(blocks, 128).T)
        # [4, (blocks//4)*128]: [k, g*128+p] = invd[(4g+k)*128+p]
        invd_b4 = np.ascontiguousarray(
            invd.reshape(blocks // 4, 4, 128).transpose(1, 0, 2)
            .reshape(4, -1))
        invd_pm = np.ascontiguousarray(invd.reshape(blocks, 128).T)
        in_maps.append({
            "x_ct": x_ct, "dis_pm": dis_pm, "dis2_pm": dis2_pm,
            "invd_b4": invd_b4, "invd_pm": invd_pm,
            "idx_plane": per_core[c]["idx_plane"],
            "rel_plane": _bf16(per_core[c]["rel_plane"]),
            "w1": w1, "w2p": w2p, "b1r": b1r, "b2d": b2d,
            "ones_row": ones_row,
            "iota": iota, "eye": eye,
        })
    return meta, in_maps


def _build_program(cfg, meta):
    import concourse.bacc as bacc
    import concourse.mybir as mybir
    from concourse import tile

    d = meta["d"]
    blocks, shard, qrows, chunk, bpp = (d["blocks"], d["shard"], d["qrows"],
                                        d["chunk"], d["bpp"])
    supers = meta["supers"]
    tiles_bq = meta["tiles_bq"]
    tot_tiles = meta["tot_tiles"]
    batches = meta["batches"]
    cin, chid, cout = cfg["CIN"], cfg["CHID"], cfg["COUT"]

    bf16 = mybir.dt.bfloat16
    f32 = mybir.dt.float32
    i16 = mybir.dt.int16
    mult = mybir.AluOpType.mult
    iseq = mybir.AluOpType.is_equal
    Relu = mybir.ActivationFunctionType.Relu
    Copy = mybir.ActivationFunctionType.Copy

    nc = bacc.Bacc("TRN2", target_bir_lowering=False, debug=False,
                   num_devices=NCORES, num_swdge_queues=4)

    x_ct = nc.dram_tensor("x_ct", [cin, shard], bf16, kind="ExternalInput")
    dis_pm_t = nc.dram_tensor("dis_pm", [128, blocks], f32,
                              kind="ExternalInput")
    dis2_pm_t = nc.dram_tensor("dis2_pm", [128, blocks], f32,
                               kind="ExternalInput")
    invd_b4_t = nc.dram_tensor("invd_b4", [4, (blocks // 4) * 128], f32,
                               kind="ExternalInput")
    invd_pm_t = nc.dram_tensor("invd_pm", [128, blocks], f32,
                               kind="ExternalInput")
    idxp_t = nc.dram_tensor("idx_plane", [128, tot_tiles * 8], i16,
                            kind="ExternalInput")
    relp_t = nc.dram_tensor("rel_plane", [128, tot_tiles], bf16,
                            kind="ExternalInput")
    w1_t = nc.dram_tensor("w1", [cin, chid], bf16, kind="ExternalInput")
    w2p_t = nc.dram_tensor("w2p", [chid, 128], bf16, kind="ExternalInput")
    b1r_t = nc.dram_tensor("b1r", [1, chid], bf16, kind="ExternalInput")
    ones_t = nc.dram_tensor("ones_row", [1, 128], bf16,
                            kind="ExternalInput")
    b2d_t = nc.dram_tensor("b2d", [4, 512], f32, kind="ExternalInput")
    iota_t = nc.dram_tensor("iota", [128, 128], bf16, kind="ExternalInput")
    eye_t = nc.dram_tensor("eye", [128, 128], bf16, kind="ExternalInput")
    out_t = nc.dram_tensor("out", [shard, cout], f32, kind="ExternalOutput")

    # Shared-scratchpad AllGather outputs (faster HBM-HBM collectives)
    tab1 = [nc.dram_tensor(f"t1_{j}", [chunk, chid], bf16,
                           addr_space="Shared") for j in range(PIECES)]
    tab2 = [nc.dram_tensor(f"t2_{j}", [chunk, 128], bf16,
                           addr_space="Shared") for j in range(PIECES)]

    with tile.TileContext(nc) as tc:
        with (
            tc.tile_pool(name="dram", bufs=1, space="DRAM") as dram,
            tc.tile_pool(name="const", bufs=1) as cp,
            tc.tile_pool(name="shards", bufs=1) as shp,
            tc.tile_pool(name="xs", bufs=4) as xp,
            tc.tile_pool(name="stage", bufs=10) as stp,
            tc.tile_pool(name="idxs", bufs=12) as ixp,
            tc.tile_pool(name="masks", bufs=5) as mp,
            tc.tile_pool(name="diag", bufs=4) as dgp,
            tc.tile_pool(name="h1t", bufs=3) as hp,
            tc.tile_pool(name="outp", bufs=4) as op_,
            tc.tile_pool(name="pbig", bufs=4, space="PSUM") as pbig,
            tc.tile_pool(name="pph1", bufs=2, space="PSUM") as pph1,
            tc.tile_pool(name="ppg", bufs=2, space="PSUM") as ppg,
        ):
            # ---- DRAM scratch (collective inputs must be Local) ----
            bounce1 = [dram.tile([qrows, chid], bf16, name=f"bo1_{j}",
                                 tag=f"bo1_{j}") for j in range(PIECES)]
            bounce2 = [dram.tile([qrows, 128], bf16, name=f"bo2_{j}",
                                 tag=f"bo2_{j}") for j in range(PIECES)]

            # ---- constants ----
            iota_sb = cp.tile([128, 128], bf16)
            nc.sync.dma_start(iota_sb[:], iota_t[:])
            eye_sb = cp.tile([128, 128], bf16)
            nc.sync.dma_start(eye_sb[:], eye_t[:])
            w1_sb = cp.tile([cin, chid], bf16)
            nc.sync.dma_start(w1_sb[:], w1_t[:])
            w2_sb = cp.tile([chid, 128], bf16)
            nc.sync.dma_start(w2_sb[:], w2p_t[:])
            b1_sb = cp.tile([1, chid], bf16)
            nc.sync.dma_start(b1_sb[:], b1r_t[:])
            b2_sb = cp.tile([4, 512], f32)
            nc.sync.dma_start(b2_sb[:], b2d_t[:])
            relp_sb = cp.tile([128, tot_tiles], bf16)
            nc.sync.dma_start(relp_sb[:], relp_t[:])

            dis_pm = cp.tile([128, blocks], f32)
            nc.sync.dma_start(dis_pm[:], dis_pm_t[:])
            dis2_pm = cp.tile([128, blocks], f32)
            nc.sync.dma_start(dis2_pm[:], dis2_pm_t[:])
            invd_b4 = cp.tile([4, (blocks // 4) * 128], f32)
            nc.sync.dma_start(invd_b4[:], invd_b4_t[:])
            invd_pm = cp.tile([128, blocks], f32)
            nc.sync.dma_start(invd_pm[:], invd_pm_t[:])
            ones_sb = cp.tile([1, 128], bf16)
            nc.sync.dma_start(ones_sb[:], ones_t[:])
            # b1 broadcast across partitions: [128, chid], row k = b1
            pb1 = pph1.tile([128, chid], f32, tag="ph1")
            nc.tensor.matmul(pb1[:], ones_sb[:], b1_sb[:],
                             start=True, stop=True)
            b1bc = cp.tile([128, chid], bf16)
            nc.scalar.activation(b1bc[:], pb1[:], Copy)

            g1s = shp.tile([128, blocks * chid], bf16)
            g2s = shp.tile([128, blocks * 128], bf16)

            def exchange(bounce, tabs, j):
                nc.gpsimd.collective_compute(
                    "AllGather", mybir.AluOpType.bypass,
                    replica_groups=[list(range(NCORES))],
                    ins=[bounce[j].opt()], outs=[tabs[j][:].opt()])

            # ---- phase 1: dense transform -> g1 shard, exchange per piece
            for j in range(PIECES):
                for b in range(j * bpp, (j + 1) * bpp):
                    xb = xp.tile([cin, 128], bf16, tag="xb")
                    nc.sync.dma_start(xb[:], x_ct[:, b * 128:(b + 1) * 128])
                    pt = pph1.tile([128, chid], f32, tag="ph1")
                    nc.tensor.matmul(pt[:], xb[:], w1_sb[:],
                                     start=True, stop=True)
                    nc.scalar.activation(
                        g1s[:, b * chid:(b + 1) * chid], pt[:], Copy,
                        bias=0.0, scale=dis_pm[:, b:b + 1])
                g1s3 = g1s[:].rearrange("p (b c) -> p b c", c=chid)
                nc.sync.dma_start(
                    bounce1[j][:].rearrange("(b p) c -> p b c", p=128),
                    g1s3[:, j * bpp:(j + 1) * bpp, :])
                exchange(bounce1, tab1, j)

            # ---- gather/aggregate layers ----
            # layer 1: psum [ch, dst]  (lhsT=st, rhs=mask)
            # layer 2: psum [dst, ch]  (lhsT=mask, rhs=st)
            l2x_done = [False] * PIECES

            def l2_exchange_ready(b_done):
                """Fire layer-2 exchanges whose block range is evicted."""
                for j in range(PIECES):
                    if not l2x_done[j] and b_done >= (j + 1) * bpp:
                        g2s3 = g2s[:].rearrange("p (b c) -> p b c", c=128)
                        nc.sync.dma_start(
                            bounce2[j][:].rearrange("(b p) c -> p b c",
                                                    p=128),
                            g2s3[:, j * bpp:(j + 1) * bpp, :])
                        exchange(bounce2, tab2, j)
                        l2x_done[j] = True

            def aggregate(layer, tabs):
                tile_cursor = 0
                batch_i = 0
                b0 = 0
                for g in supers:
                    assert g % 4 == 0
                    nbank = g // 4
                    psums = [pbig.tile([128, 512], f32, name="acc",
                                       tag="acc") for _ in range(nbank)]

                    def pacc(bi):
                        return psums[bi // 4][:, (bi % 4) * 128:
                                              (bi % 4) * 128 + 128]

                    # program-order matmul sequence; find last item per bank
                    seq = [("self", bi) for bi in range(g)]
                    for qq in range(PIECES):
                        for bi in range(g):
                            nt = int(tiles_bq[(b0 + bi) * PIECES + qq])
                            for t in range(nt):
                                seq.append(("edge", qq, bi, t))
                    last_per_bank = {}
                    for item in seq:
                        bi = item[1] if item[0] == "self" else item[2]
                        last_per_bank[bi // 4] = item

                    # seeds
                    for k in range(nbank):
                        gb = (b0 + k * 4) // 4  # global bank index
                        if layer == 1:
                            # psum[ch, dst region] = b1[ch] * invd[dst]
                            for kk in range(4):
                                b = b0 + k * 4 + kk
                                dg = dgp.tile([128, 128], bf16, tag="dg")
                                nc.vector.tensor_scalar(
                                    dg[:], eye_sb[:],
                                    invd_pm[:, b:b + 1], None, mult)
                                nc.tensor.matmul(
                                    psums[k][:, kk * 128:(kk + 1) * 128],
                                    b1bc[:], dg[:],
                                    start=True, stop=False)
                        else:
                            # psum[dst, ch4] = invd[dst] * b2 blockdiag
                            nc.tensor.matmul(
                                psums[k][:],
                                invd_b4[:, gb * 128:(gb + 1) * 128],
                                b2_sb[:], start=True, stop=False)
                    # self loops
                    for bi in range(g):
                        b = b0 + bi
                        stop = last_per_bank[bi // 4] == ("self", bi)
                        if layer == 1:
                            nc.tensor.matmul(
                                pacc(bi), g1s[:, b * chid:(b + 1) * chid],
                                eye_sb[:], start=False, stop=stop)
                        else:
                            nc.tensor.matmul(
                                pacc(bi), eye_sb[:],
                                g2s[:, b * 128:(b + 1) * 128],
                                start=False, stop=stop)
                    # edge tiles, batched per source piece
                    for qq in range(PIECES):
                        nb = batches[batch_i]
                        batch_i += 1
                        if nb == 0:
                            continue
                        idxb = ixp.tile([128, nb * 8], i16, tag="idxb")
                        nc.sync.dma_start(
                            idxb[:], idxp_t[:, tile_cursor * 8:
                                            (tile_cursor + nb) * 8])
                        st = stp.tile([128, nb, 128], bf16, tag="stage")
                        nc.gpsimd.dma_gather(
                            st[:], tabs[qq][:], idxb[:],
                            nb * 128, nb * 128, 128,
                            single_packet=False, queue_num=qq % 4)
                        # one wide mask build for the whole batch
                        mk = mp.tile([128, nb, 128], bf16, tag="mask")
                        iota_b = iota_sb[:].rearrange(
                            "p (t c) -> p t c", t=1).broadcast_to(
                                [128, nb, 128])
                        rel_b = relp_sb[:, tile_cursor:
                                        tile_cursor + nb].rearrange(
                            "p (t o) -> p t o", o=1).broadcast_to(
                                [128, nb, 128])
                        nc.vector.tensor_tensor(mk[:], iota_b, rel_b, iseq)
                        t_local = 0
                        for bi in range(g):
                            b = b0 + bi
                            nt = int(tiles_bq[b * PIECES + qq])
                            for t in range(nt):
                                stop = (last_per_bank[bi // 4] ==
                                        ("edge", qq, bi, t))
                                st_t = st[:, t_local, :].squeeze()
                                mk_t = mk[:, t_local, :].squeeze()
                                if layer == 1:
                                    nc.tensor.matmul(pacc(bi), st_t, mk_t,
                                                     start=False, stop=stop)
                                else:
                                    nc.tensor.matmul(pacc(bi), mk_t, st_t,
                                                     start=False, stop=stop)
                                t_local += 1
                        tile_cursor += nb
                    # evictions
                    if layer == 1:
                        for k in range(nbank):
                            h1b = hp.tile([128, 512], bf16, tag="h1b")
                            nc.scalar.activation(h1b[:], psums[k][:], Relu)
                            for kk in range(4):
                                bi = k * 4 + kk
                                b = b0 + bi
                                pg = ppg.tile([128, 128], f32, tag="pg")
                                nc.tensor.matmul(
                                    pg[:], h1b[:, kk * 128:(kk + 1) * 128],
                                    w2_sb[:], start=True, stop=True)
                                nc.scalar.activation(
                                    g2s[:, b * 128:(b + 1) * 128], pg[:],
                                    Copy, bias=0.0,
                                    scale=dis2_pm[:, b:b + 1])
                        l2_exchange_ready(b0 + g)
                    else:
                        for bi in range(g):
                            b = b0 + bi
                            ob = op_.tile([128, cout], f32, tag="ob")
                            nc.scalar.activation(
                                ob[:], pacc(bi)[:, :cout], Copy, bias=0.0,
                                scale=dis_pm[:, b:b + 1])
                            nc.sync.dma_start(
                                out_t[b * 128:(b + 1) * 128, :], ob[:])
                    b0 += g

            aggregate(1, tab1)
            aggregate(2, tab2)

    nc.compile()
    return nc


def run_config(inputs, cfg, run=None):
    from concourse.bass_utils import run_bass_kernel_spmd

    x = np.asarray(inputs["x"], np.float32)
    edge_index = np.asarray(inputs["edge_index"])
    meta, in_maps = _host_inputs(
        x, edge_index, inputs["W1"], inputs["b1"], inputs["W2"],
        inputs["b2"], cfg)
    nc = _build_program(cfg, meta)
    if run is None:
        def run(nc, in_maps):
            return run_bass_kernel_spmd(
                nc, in_maps, list(range(NCORES))).results
    results = run(nc, in_maps)
    d = _derive(cfg)
    shard = d["shard"]
    all_rows = np.concatenate(
        [results[c]["out"] for c in range(NCORES)], axis=0)
    node_of_slot = meta["node_of_slot"]
    valid = node_of_slot >= 0
    out = np.empty((cfg["N"], cfg["COUT"]), np.float32)
    out[node_of_slot[valid]] = all_rows[valid]
    return np.ascontiguousarray(out)


def kernel(**inputs):
    return run_config(inputs, CFG_FULL)
